# revision 34
# baseline (speedup 1.0000x reference)
"""BNT Channel Attention kernel for 8x TRN2 NeuronCores.

Reference computation (per batch b of 8, one batch per core):
    qkv = x @ W_qkv + b_qkv            # [4096, 3072]
    q, k, v = split(qkv)               # each [4096, 1024], 16 heads x 64
    attn_h = softmax((q_h^T @ k_h) / 8, axis=-1)   # [64, 64] per head
    out_h  = v_h @ attn_h              # [4096, 64]
    out    = concat_h(out_h)           # [4096, 1024]

Strategy (v7 - Gram factoring):
- Data parallel over batch: core c handles batch c (no collectives).
- Q^T K = Wq^T (x^T x) Wk + bias cross-terms.  The Gram matrix
  G = x^T x (contract over N=4096) exploits symmetry: only the
  upper-triangle tile stripes are computed, the lower tiles come from
  25 cheap PE transposes.  Then U = G @ Wk and per-head-pair
  A = Wq^T U (junk-packed to N=256 for f32r full speed).  Total logit
  cost ~239K PE cycles vs 590K for explicit Q,K projections.
- Bias cross-terms (sq bk^T + bq sk^T + N bq bk^T, s = x.sum(0)) are
  computed on HOST and preloaded into the SBUF logit accumulator, with
  -1e30 in the junk quadrants so exp() zeroes them and the activation
  accum_out row-sum is exact.
- PSUM discipline (hardware: one open accumulation group per 2KB bank,
  matmul out never crosses a bank): pass-1 stripes own two banks each
  (8 banks); pass 2 runs in a second pool (4 stripes x 1 bank + 1
  transpose bank); U owns 2x2 banks (bufs=2); the per-(di,pair) A
  matmuls are closed groups drained to SBUF by strided DVE adds.
- x tiles 0..22 stay resident (92KB/partition); tiles 23..31 stream
  through a 3-buf ring.  Pass 2 re-reads cols 512:1024 only: tiles
  29..31 straight from the still-live ring buffers, 23..28 via a small
  second ring prefetched on the idle sync queue, resident tiles last.
  The 92KB x slot is recycled (same pool tag) as the post-Gram arena
  holding Wq, Wv', the xT ring, the U ring and bv'.
- Softmax skips the max-shift (|logits/8| < 45 for randn data): per
  pair, Act-engine exp with accum_out gives the row sum for free, DVE
  reciprocal + a broadcast mul write the bf16 block-diagonal attn.
- V path by associativity: out = x @ (Wv @ attn) + 1 (bv @ attn), with
  attn/Wv in bf16 (output-linear precision, junk-free N=128 matmuls
  run full speed in bf16).  Pass B re-streams host-transposed xT.
"""

import numpy as np
import ml_dtypes

import concourse.bacc as bacc
import concourse.bass as bass
import concourse.mybir as mybir
import concourse.tile as tile
from concourse import bass_utils

B = 8
NSEQ = 4096
D = 1024
H = 16
DH = 64
NPAIR = 8          # head pairs (2 heads = 128 channels per pair)
P = 128
KT = D // P        # 8 k-tiles over the D contraction
NT = NSEQ // P     # 32 Gram N-tiles
RES = 23           # x tiles 0..22 resident; 23..31 ring-streamed
CHUNK = 256        # pass-B rows per xT chunk
NCHUNK = NSEQ // CHUNK
MPC = CHUNK // P   # 2 row-tiles per chunk

F32 = mybir.dt.float32
F32R = mybir.dt.float32r
BF16 = mybir.dt.bfloat16

# Gram stripes: (di, psum_off, xcol_start, piece widths).  Stripe di
# holds G[di-tile rows, xcol_start..1024).  Every piece is >=256 wide
# (f32r full speed), sits inside one 2KB PSUM bank, and each stripe
# owns its banks exclusively (accumulation groups stay open over the
# whole 32-tile loop, and the zero-region is bank-granular).
P1S = [
    (0, 0,   ((512, 0), (512, 512))),
    (1, 128, ((512, 1024), (384, 1536))),
    (2, 256, ((512, 2048), (256, 2560))),
    (3, 384, ((256, 3072), (384, 3584))),
]
P2S = [
    (4, 512, ((512, 0),)),
    (5, 640, ((384, 512),)),
    (6, 768, ((256, 1024),)),
    (7, 768, ((256, 1536),)),
]
# gsb slot (k, m) = G[k-rows, m-cols] tile at col offset (8k+m)*128
DIRECT = {(di, j) for di, cs, _ in P1S + P2S for j in range(cs // P, KT)}
MISSING = [(a, b) for a in range(KT) for b in range(KT)
           if (a, b) not in DIRECT]          # 27 tiles, all with a > b
EARLY_T = [(a, b) for a, b in MISSING if b <= 3]   # sources in pass 1
LATE_T = [(a, b) for a, b in MISSING if b > 3]     # need s4/s5 copies

# arena (f32 words): recycles the 92KB x slot after the Gram
WQ_OFF = 0                  # Wq   [128, 8*1024]
WVP_OFF = 8192              # Wv'  [128, 8*1024]
XT_OFF = 16384              # xT ring: 2 x [128, 8*256]
USB_OFF = 20480             # U ring:  2 x [128, 1024]
BVP_OFF = 22528             # bv'  [1, 1024] (row replicated later)
ARENA = 23552
XRES = RES * D              # 23552, exact match

NEG = -1.0e30               # exp(NEG/8) == 0: kills junk quadrants

_CACHE = {}
_LAST_RESULTS = None


def _build():
    nc = bacc.Bacc(
        "TRN2", target_bir_lowering=False, debug=False, num_devices=B
    )
    x_d = nc.dram_tensor("x", [NSEQ, D], F32R, kind="ExternalInput").ap()
    xt_d = nc.dram_tensor("xt", [D, NSEQ], F32R, kind="ExternalInput").ap()
    wk_d = nc.dram_tensor("wk", [D, D], F32R, kind="ExternalInput").ap()
    wq_d = nc.dram_tensor("wq", [D, D], F32R, kind="ExternalInput").ap()
    wvt_d = nc.dram_tensor("wvt", [P, NPAIR * D], BF16, kind="ExternalInput").ap()
    bv_d = nc.dram_tensor("bv", [P, NPAIR], BF16, kind="ExternalInput").ap()
    c_d = nc.dram_tensor("cbias", [P, NPAIR * P], BF16, kind="ExternalInput").ap()
    eye_d = nc.dram_tensor("eye", [P, P], F32R, kind="ExternalInput").ap()
    ones_d = nc.dram_tensor("ones", [1, P], F32R, kind="ExternalInput").ap()
    out_d = nc.dram_tensor("out", [NSEQ, D], F32, kind="ExternalOutput").ap()

    x_v = x_d.rearrange("(n p) d -> p n d", p=P)     # [128, 32, 1024]
    wk_v = wk_d.rearrange("(t p) n -> p t n", p=P)   # [128, 8, 1024]
    wq_v = wq_d.rearrange("(t p) n -> p t n", p=P)
    xt_v = xt_d.rearrange("(t p) r -> p t r", p=P)   # [128, 8, 4096]

    with tile.TileContext(nc) as tc:
        with (
            tc.tile_pool(name="const", bufs=1) as cpool,
            tc.tile_pool(name="big", bufs=1) as bigpool,
            tc.tile_pool(name="ring", bufs=3) as ringpool,
            tc.tile_pool(name="ring2", bufs=3) as ring2pool,
            tc.tile_pool(name="wk", bufs=1) as wkpool,
            tc.tile_pool(name="gsb", bufs=1) as gsbpool,
            tc.tile_pool(name="wvt", bufs=1) as wvtpool,
            tc.tile_pool(name="sm", bufs=1) as smpool,
            tc.tile_pool(name="osb", bufs=2) as opool,
        ):
            # tiny consts on the sync queue (needed from the transposes on)
            eye = cpool.tile([P, P], F32R, tag="eye")
            nc.sync.dma_start(eye[:], eye_d)
            ones = cpool.tile([1, P], F32R, tag="ones")
            nc.sync.dma_start(ones[:], ones_d)

            # resident x tiles 0..22 stream in first on the gpsimd queue;
            # everything else on that queue lines up behind them
            xres = bigpool.tile([P, XRES], F32R, tag="big")
            # tile 0 in two halves so the first Gram matmul starts sooner
            nc.gpsimd.dma_start(xres[:, 0:512], x_v[:, 0, 0:512])
            nc.gpsimd.dma_start(xres[:, 512:D], x_v[:, 0, 512:D])
            for n in range(1, RES):
                nc.gpsimd.dma_start(
                    xres[:, n * D : (n + 1) * D], x_v[:, n, :]
                )

            wk_sb = wkpool.tile([P, KT * D], F32R, tag="wk")
            gsb = gsbpool.tile([P, KT * KT * P], F32R, tag="gsb")
            wvt = wvtpool.tile([P, NPAIR * D], BF16, tag="wvt")
            bv = cpool.tile([P, NPAIR], BF16, tag="bv")
            c_sb = cpool.tile([P, NPAIR * P], BF16, tag="cbias")
            attn_acc = smpool.tile([P, NPAIR * P], F32, tag="attn_acc")
            bd = smpool.tile([P, NPAIR * P], BF16, tag="bd")
            rsum = smpool.tile([P, NPAIR], F32, tag="rsum")
            rinv = smpool.tile([P, NPAIR], F32, tag="rinv")
            nms = smpool.tile([P, NPAIR], F32, tag="nms")

            # PSUM->SBUF copies alternate DVE / Act (GPSIMD cannot touch
            # PSUM) so neither engine's latency gates the next PE phase
            def copy3(i, dst, src):
                if i % 2 == 0:
                    nc.vector.tensor_copy(dst, src)
                else:
                    nc.scalar.activation(
                        dst, src, mybir.ActivationFunctionType.Copy
                    )

            ring_t = {}

            # ============ Gram pass 1: stripes 0..3, two banks each
            with tc.tile_pool(name="psg1", bufs=1, space="PSUM") as psg1pool:
                ps1 = psg1pool.tile([P, 4096], F32, tag="g1")

                def ring_dma(m):
                    rt = ringpool.tile([P, D], F32R, tag="xr", name=f"xr{m}")
                    nc.gpsimd.dma_start(rt[:], x_v[:, m, :])
                    ring_t[m] = rt

                for m in range(RES, min(RES + 3, NT)):
                    ring_dma(m)
                for n in range(NT):
                    xn = (
                        xres[:, n * D : (n + 1) * D]
                        if n < RES
                        else ring_t[n][:]
                    )
                    for di, cs, pieces in P1S:
                        o = 0
                        for w, po in pieces:
                            nc.tensor.matmul(
                                ps1[:, po : po + w],
                                xn[:, di * P : (di + 1) * P],
                                xn[:, cs + o : cs + o + w],
                                start=(n == 0),
                                stop=(n == NT - 1),
                            )
                            o += w
                    if RES + 3 <= n + 3 < NT:
                        ring_dma(n + 3)
                # Wk: needed from U on; FIFO places it after the x stream
                for t in range(KT):
                    nc.gpsimd.dma_start(
                        wk_sb[:, t * D : (t + 1) * D], wk_v[:, t, :]
                    )
                # stripe copies split across DVE and Act
                nc.vector.tensor_copy(gsb[:, 0:512], ps1[:, 0:512])
                nc.scalar.activation(
                    gsb[:, 512:1024], ps1[:, 512:1024],
                    mybir.ActivationFunctionType.Copy,
                )
                # s1 -> slots (1,1..7) at gsb 1152
                nc.vector.tensor_copy(gsb[:, 1152:1664], ps1[:, 1024:1536])
                nc.scalar.activation(
                    gsb[:, 1664:2048], ps1[:, 1536:1920],
                    mybir.ActivationFunctionType.Copy,
                )
                # s2 -> slots (2,2..7) at gsb 2304
                nc.vector.tensor_copy(gsb[:, 2304:2816], ps1[:, 2048:2560])
                nc.scalar.activation(
                    gsb[:, 2816:3072], ps1[:, 2560:2816],
                    mybir.ActivationFunctionType.Copy,
                )
                # s3 -> slots (3,3..7) at gsb 3456 (pieces at ps 3072/3584)
                nc.vector.tensor_copy(gsb[:, 3456:3712], ps1[:, 3072:3328])
                nc.scalar.activation(
                    gsb[:, 3712:4096], ps1[:, 3584:3968],
                    mybir.ActivationFunctionType.Copy,
                )

            # ============ Gram pass 2: stripes 4..7 (one bank each) with
            # the pass-1-sourced transposes interleaved into the PE stream
            with tc.tile_pool(name="psg2", bufs=1, space="PSUM") as psg2pool:
                ps2 = psg2pool.tile([P, 2048], F32, tag="g2")
                tsl = psg2pool.tile([P, 4 * P], F32R, tag="t")
                ring2_t = {}

                def ring2_dma(m):
                    rt = ring2pool.tile([P, 512], F32R, tag="xr2", name=f"xr2_{m}")
                    nc.sync.dma_start(rt[:], x_v[:, m, 512:1024])
                    ring2_t[m] = rt

                for m in range(RES, min(RES + 3, NT)):
                    ring2_dma(m)
                n_t = 0

                def transpose_slot(a, b, i):
                    src = (KT * b + a) * P
                    dst = (KT * a + b) * P
                    sl = tsl[:, (i % 4) * P : (i % 4 + 1) * P]
                    nc.tensor.transpose(sl, gsb[:, src : src + P], eye[:])
                    copy3(i, gsb[:, dst : dst + P], sl)

                # ring tiles first: 29..31 still sit in the pass-1 ring
                # buffers (last 3 generations, never overwritten), 23..28
                # re-fetched (cols 512:1024) on the idle sync queue, and
                # the resident tail then runs with zero DMA dependence
                n_order = (
                    list(range(RES + 6, NT))
                    + list(range(RES, RES + 6))
                    + list(range(RES))
                )
                for idx, n in enumerate(n_order):
                    if n < RES:
                        xn = xres[:, n * D + 512 : (n + 1) * D]
                    elif n >= RES + 6:
                        xn = ring_t[n][:, 512:1024]
                    else:
                        xn = ring2_t[n][:]
                    for di, cs, pieces in P2S:
                        o = 0
                        for w, po in pieces:
                            nc.tensor.matmul(
                                ps2[:, po : po + w],
                                xn[:, di * P - 512 : (di + 1) * P - 512],
                                xn[:, cs + o - 512 : cs + o + w - 512],
                                start=(idx == 0),
                                stop=(idx == NT - 1),
                            )
                            o += w
                    if RES <= n <= RES + 2:
                        ring2_dma(n + 3)
                    if n_t < len(EARLY_T):
                        a, b = EARLY_T[n_t]
                        transpose_slot(a, b, n_t)
                        n_t += 1
                while n_t < len(EARLY_T):
                    a, b = EARLY_T[n_t]
                    transpose_slot(a, b, n_t)
                    n_t += 1
                # s4/s5 copies first (they gate the late transposes);
                # s6/s7 trail after (only needed from U6 on)
                nc.vector.tensor_copy(
                    gsb[:, (KT * 4 + 4) * P : (KT * 4 + 6) * P], ps2[:, 0:256]
                )
                nc.scalar.activation(
                    gsb[:, (KT * 4 + 6) * P : (KT * 4 + 8) * P],
                    ps2[:, 256:512],
                    mybir.ActivationFunctionType.Copy,
                )
                nc.vector.tensor_copy(
                    gsb[:, (KT * 5 + 5) * P : (KT * 5 + 8) * P],
                    ps2[:, 512:896],
                )
                for a, b in LATE_T:
                    transpose_slot(a, b, n_t)
                    n_t += 1
                nc.vector.tensor_copy(
                    gsb[:, (KT * 6 + 6) * P : (KT * 6 + 8) * P],
                    ps2[:, 1024:1280],
                )
                nc.scalar.activation(
                    gsb[:, (KT * 7 + 6) * P : (KT * 7 + 8) * P],
                    ps2[:, 1536:1792],
                    mybir.ActivationFunctionType.Copy,
                )

            # arena recycles the x slot; Wq lands during U (WAR-gated)
            arena = bigpool.tile([P, ARENA], F32R, tag="big")
            for t in range(KT):
                nc.gpsimd.dma_start(
                    arena[:, WQ_OFF + t * D : WQ_OFF + (t + 1) * D],
                    wq_v[:, t, :],
                )
            nc.gpsimd.dma_start(wvt[:], wvt_d)
            nc.gpsimd.dma_start(c_sb[:], c_d)
            nc.gpsimd.dma_start(bv[:], bv_d)
            # logit accumulator starts as the host bias correction C
            # (junk quadrants hold -1e30 so exp() zeroes them later)
            nc.vector.tensor_copy(attn_acc[:], c_sb[:])

            # ============ U = G @ Wk per di-stripe; A = Wq^T U as closed
            # per-(di,pair) PSUM groups drained into attn_acc by DVE
            with (
                tc.tile_pool(name="psu", bufs=2, space="PSUM") as psupool,
                tc.tile_pool(name="psa", bufs=1, space="PSUM") as psapool,
            ):
                ac_j = attn_acc[:].rearrange("q (j t) -> q j t", j=4)

                def emit_U(di):
                    psu = psupool.tile([P, D], F32, tag="u", name=f"psu{di}")
                    for k in range(KT):
                        g0 = (KT * k + di) * P
                        for h2 in range(2):
                            nc.tensor.matmul(
                                psu[:, h2 * 512 : (h2 + 1) * 512],
                                gsb[:, g0 : g0 + P],
                                wk_sb[:, k * D + h2 * 512 : k * D + (h2 + 1) * 512],
                                start=(k == 0),
                                stop=(k == KT - 1),
                            )
                    u0 = USB_OFF + (di % 2) * D
                    copy3(di, arena[:, u0 : u0 + D], psu[:])

                def emit_A(di):
                    u0 = USB_OFF + (di % 2) * D
                    ps_a = psapool.tile(
                        [P, 2 * NPAIR * P], F32, tag="a", name=f"psa{di}"
                    )
                    for p in range(NPAIR):
                        j = p // 2
                        nc.tensor.matmul(
                            ps_a[:, 256 * p : 256 * (p + 1)],
                            arena[:, WQ_OFF + di * D + P * p : WQ_OFF + di * D + P * (p + 1)],
                            arena[:, u0 + 256 * j : u0 + 256 * (j + 1)],
                            start=True,
                            stop=True,
                        )
                    # drain the diag halves into the SBUF accumulator:
                    # even pairs at ps[512j+0], odd at ps[512j+384]
                    ps_j = ps_a[:].rearrange("q (j t) -> q j t", j=4)
                    for par in range(2):
                        nc.vector.tensor_add(
                            ac_j[:, :, P * par : P * (par + 1)],
                            ac_j[:, :, P * par : P * (par + 1)],
                            ps_j[:, :, 384 * par : 384 * par + P],
                        )

                # software-pipelined emission: A(di) after U(di+1) so the
                # in-order PE stream works on U(di+1) while usb(di)'s
                # PSUM->SBUF copy completes
                emit_U(0)
                for di in range(1, KT):
                    emit_U(di)
                    emit_A(di - 1)
                emit_A(KT - 1)

            # ============ Softmax per pair: DVE row-max (as the exp's
            # per-partition bias, scaled) -> Act exp with accum_out row
            # sum -> DVE reciprocal -> broadcast mul into bf16 attn.
            # Junk quadrants carry -1e30 logits: never the max, exp to 0,
            # so row sums and the bf16 block-diagonal come out exact.
            for p in range(NPAIR):
                blk = slice(P * p, P * (p + 1))
                nc.vector.reduce_max(
                    nms[:, p : p + 1], attn_acc[:, blk],
                    axis=mybir.AxisListType.X, negate=True,
                )
                nc.vector.tensor_scalar_mul(
                    nms[:, p : p + 1], nms[:, p : p + 1], 0.125
                )
                nc.scalar.activation(
                    attn_acc[:, blk], attn_acc[:, blk],
                    mybir.ActivationFunctionType.Exp,
                    scale=0.125,
                    bias=nms[:, p : p + 1],
                    accum_out=rsum[:, p : p + 1],
                )
                nc.vector.reciprocal(rinv[:, p : p + 1], rsum[:, p : p + 1])
                eng = nc.vector if p % 2 == 0 else nc.gpsimd
                eng.tensor_mul(
                    bd[:, blk],
                    attn_acc[:, blk],
                    rinv[:, p : p + 1].broadcast_to([P, P]),
                )

            # ============ Wv' = Wv @ attn first (gates pass B), then bv'
            with tc.tile_pool(name="psw", bufs=3, space="PSUM") as pswpool:
                # bf16 lets the per-pair N=128 matmuls run at 1 cyc/row
                for t in range(KT):
                    psw = pswpool.tile([P, D], F32, tag="w")
                    for p in range(NPAIR):
                        nc.tensor.matmul(
                            psw[:, P * p : P * (p + 1)],
                            wvt[:, D * p + t * P : D * p + (t + 1) * P],
                            bd[:, P * p : P * (p + 1)],
                            start=True,
                            stop=True,
                        )
                    copy3(
                        t,
                        arena[:, WVP_OFF + t * D : WVP_OFF + (t + 1) * D],
                        psw[:],
                    )
                # bv' = bv @ attn, replicated across partitions into
                # attn_acc (dead after the muls) for the pass-B bias adds
                ps_bv = pswpool.tile([1, D], F32, tag="bvp", bufs=1)
                for p in range(NPAIR):
                    nc.tensor.matmul(
                        ps_bv[:, P * p : P * (p + 1)],
                        bv[:, p : p + 1],
                        bd[:, P * p : P * (p + 1)],
                        start=True,
                        stop=True,
                    )
                bvp = arena[0:1, BVP_OFF : BVP_OFF + D]
                nc.vector.tensor_copy(bvp, ps_bv[:])
                ps_br = pswpool.tile([P, D], F32, tag="w")
                for h2 in range(2):
                    nc.tensor.matmul(
                        ps_br[:, h2 * 512 : (h2 + 1) * 512],
                        ones[:],
                        bvp[:, h2 * 512 : (h2 + 1) * 512],
                        start=True,
                        stop=True,
                    )
                nc.vector.tensor_copy(attn_acc[:], ps_br[:])

            # ============ Pass B: out = x @ Wv' + bv'
            with tc.tile_pool(name="pso", bufs=2, space="PSUM") as psopool:
                for ch in range(NCHUNK):
                    x0 = XT_OFF + (ch % 2) * (KT * CHUNK)
                    xt_sb = arena[:, x0 : x0 + KT * CHUNK]
                    nc.gpsimd.dma_start(
                        xt_sb.rearrange("p (t r) -> p t r", t=KT),
                        xt_v[:, :, ch * CHUNK : (ch + 1) * CHUNK],
                    )
                    for mi in range(MPC):
                        m = ch * MPC + mi
                        ps_o = psopool.tile([P, D], F32, tag="o")
                        for k in range(KT):
                            for h2 in range(2):
                                nc.tensor.matmul(
                                    ps_o[:, h2 * 512 : (h2 + 1) * 512],
                                    xt_sb[:, CHUNK * k + mi * P : CHUNK * k + (mi + 1) * P],
                                    arena[:, WVP_OFF + D * k + 512 * h2 : WVP_OFF + D * k + 512 * (h2 + 1)],
                                    start=(k == 0),
                                    stop=(k == KT - 1),
                                )
                        out_sb = opool.tile([P, D], F32, tag="osb")
                        if m == NSEQ // P - 1:
                            # split the last store so its DMA overlaps the
                            # second half's add (shaves the tail drain)
                            nc.vector.tensor_add(
                                out_sb[:, 0:512], ps_o[:, 0:512],
                                attn_acc[:, 0:512],
                            )
                            nc.scalar.dma_start(
                                out_d[m * P : (m + 1) * P, 0:512],
                                out_sb[:, 0:512],
                            )
                            nc.vector.tensor_add(
                                out_sb[:, 512:D], ps_o[:, 512:D],
                                attn_acc[:, 512:D],
                            )
                            nc.sync.dma_start(
                                out_d[m * P : (m + 1) * P, 512:D],
                                out_sb[:, 512:D],
                            )
                        else:
                            nc.vector.tensor_add(
                                out_sb[:], ps_o[:], attn_acc[:]
                            )
                            nc.scalar.dma_start(
                                out_d[m * P : (m + 1) * P, :], out_sb[:]
                            )

    nc.compile()
    return nc


def host_inputs(x, W_qkv, b_qkv):
    """Per-core input maps (host prep: transposes, packing, bias C)."""
    bf16 = ml_dtypes.bfloat16
    wvt = np.ascontiguousarray(
        W_qkv[:, 2 * D :].T.reshape(NPAIR, P, D).transpose(1, 0, 2)
        .reshape(P, NPAIR * D)
    ).astype(bf16)
    bv = np.ascontiguousarray(
        b_qkv[2 * D :].reshape(NPAIR, P).T
    ).astype(bf16)
    eye = np.eye(P, dtype=np.float32)
    ones = np.ones((1, P), np.float32)
    bq = b_qkv[:D]
    bk = b_qkv[D : 2 * D]

    in_maps = []
    for c in range(B):
        s = x[c].sum(axis=0, dtype=np.float64).astype(np.float32)
        sq = s @ W_qkv[:, :D]
        sk = s @ W_qkv[:, D : 2 * D]
        cpk = np.full((P, NPAIR * P), NEG, np.float32)
        for p in range(NPAIR):
            r = slice(P * p, P * (p + 1))
            sub = (
                np.outer(sq[r], bk[r])
                + np.outer(bq[r], sk[r])
                + float(NSEQ) * np.outer(bq[r], bk[r])
            )
            sub[:DH, DH:] = NEG
            sub[DH:, :DH] = NEG
            cpk[:, r] = sub
        in_maps.append(
            {
                "x": x[c],
                "xt": np.ascontiguousarray(x[c].T),
                "wk": np.ascontiguousarray(W_qkv[:, D : 2 * D]),
                "wq": np.ascontiguousarray(W_qkv[:, :D]),
                "wvt": wvt,
                "bv": bv,
                "cbias": cpk.astype(bf16),
                "eye": eye,
                "ones": ones,
            }
        )
    return in_maps


def kernel(x, W_qkv, b_qkv):
    global _LAST_RESULTS
    x = np.ascontiguousarray(x, dtype=np.float32)
    W_qkv = np.ascontiguousarray(W_qkv, dtype=np.float32)
    b_qkv = np.ascontiguousarray(b_qkv, dtype=np.float32)

    if "nc" not in _CACHE:
        _CACHE["nc"] = _build()
    nc = _CACHE["nc"]

    res = bass_utils.run_bass_kernel_spmd(
        nc, host_inputs(x, W_qkv, b_qkv), core_ids=list(range(B))
    )
    _LAST_RESULTS = res
    return np.stack([r["out"] for r in res.results], axis=0)


# revision 37
# speedup vs baseline: 1.0025x; 1.0025x over previous
"""BNT Channel Attention kernel for 8x TRN2 NeuronCores.

Reference computation (per batch b of 8, one batch per core):
    qkv = x @ W_qkv + b_qkv            # [4096, 3072]
    q, k, v = split(qkv)               # each [4096, 1024], 16 heads x 64
    attn_h = softmax((q_h^T @ k_h) / 8, axis=-1)   # [64, 64] per head
    out_h  = v_h @ attn_h              # [4096, 64]
    out    = concat_h(out_h)           # [4096, 1024]

Strategy (v7 - Gram factoring):
- Data parallel over batch: core c handles batch c (no collectives).
- Q^T K = Wq^T (x^T x) Wk + bias cross-terms.  The Gram matrix
  G = x^T x (contract over N=4096) exploits symmetry: only the
  upper-triangle tile stripes are computed, the lower tiles come from
  25 cheap PE transposes.  Then U = G @ Wk and per-head-pair
  A = Wq^T U (junk-packed to N=256 for f32r full speed).  Total logit
  cost ~239K PE cycles vs 590K for explicit Q,K projections.
- Bias cross-terms (sq bk^T + bq sk^T + N bq bk^T, s = x.sum(0)) are
  computed on HOST and preloaded into the SBUF logit accumulator, with
  -1e30 in the junk quadrants so exp() zeroes them and the activation
  accum_out row-sum is exact.
- PSUM discipline (hardware: one open accumulation group per 2KB bank,
  matmul out never crosses a bank): pass-1 stripes own two banks each
  (8 banks); pass 2 runs in a second pool (4 stripes x 1 bank + 1
  transpose bank); U owns 2x2 banks (bufs=2); the per-(di,pair) A
  matmuls are closed groups drained to SBUF by strided DVE adds.
- x tiles 0..22 stay resident (92KB/partition); tiles 23..31 stream
  through a 3-buf ring.  Pass 2 re-reads cols 512:1024 only: tiles
  29..31 straight from the still-live ring buffers, 23..28 via a small
  second ring prefetched on the idle sync queue, resident tiles last.
  The 92KB x slot is recycled (same pool tag) as the post-Gram arena
  holding Wq, Wv', the xT ring, the U ring and bv'.
- Softmax skips the max-shift (|logits/8| < 45 for randn data): per
  pair, Act-engine exp with accum_out gives the row sum for free, DVE
  reciprocal + a broadcast mul write the bf16 block-diagonal attn.
- V path by associativity: out = x @ (Wv @ attn) + 1 (bv @ attn), with
  attn/Wv in bf16 (output-linear precision, junk-free N=128 matmuls
  run full speed in bf16).  Pass B re-streams host-transposed xT.
"""

import numpy as np
import ml_dtypes

import concourse.bacc as bacc
import concourse.bass as bass
import concourse.mybir as mybir
import concourse.tile as tile
from concourse import bass_utils

B = 8
NSEQ = 4096
D = 1024
H = 16
DH = 64
NPAIR = 8          # head pairs (2 heads = 128 channels per pair)
P = 128
KT = D // P        # 8 k-tiles over the D contraction
NT = NSEQ // P     # 32 Gram N-tiles
RES = 23           # x tiles 0..22 resident; 23..31 ring-streamed
CHUNK = 256        # pass-B rows per xT chunk
NCHUNK = NSEQ // CHUNK
MPC = CHUNK // P   # 2 row-tiles per chunk

F32 = mybir.dt.float32
F32R = mybir.dt.float32r
BF16 = mybir.dt.bfloat16

# Gram stripes: (di, psum_off, xcol_start, piece widths).  Stripe di
# holds G[di-tile rows, xcol_start..1024).  Every piece is >=256 wide
# (f32r full speed), sits inside one 2KB PSUM bank, and each stripe
# owns its banks exclusively (accumulation groups stay open over the
# whole 32-tile loop, and the zero-region is bank-granular).
P1S = [
    (0, 0,   ((512, 0), (512, 512))),
    (1, 128, ((512, 1024), (384, 1536))),
    (2, 256, ((512, 2048), (256, 2560))),
    (3, 384, ((256, 3072), (384, 3584))),
]
P2S = [
    (4, 512, ((512, 0),)),
    (5, 640, ((384, 512),)),
    (6, 768, ((256, 1024),)),
    (7, 768, ((256, 1536),)),
]
# gsb slot (k, m) = G[k-rows, m-cols] tile at col offset (8k+m)*128
DIRECT = {(di, j) for di, cs, _ in P1S + P2S for j in range(cs // P, KT)}
MISSING = [(a, b) for a in range(KT) for b in range(KT)
           if (a, b) not in DIRECT]          # 27 tiles, all with a > b
EARLY_T = [(a, b) for a, b in MISSING if b <= 3]   # sources in pass 1
LATE_T = [(a, b) for a, b in MISSING if b > 3]     # need s4/s5 copies

# arena (f32 words): recycles the 92KB x slot after the Gram
WQ_OFF = 0                  # Wq   [128, 8*1024]
WVP_OFF = 8192              # Wv'  [128, 8*1024]
XT_OFF = 16384              # xT ring: 2 x [128, 8*256]
USB_OFF = 20480             # U ring:  2 x [128, 1024]
BVP_OFF = 22528             # bv'  [1, 1024] (row replicated later)
ARENA = 23552
XRES = RES * D              # 23552, exact match

NEG = -1.0e30               # exp(NEG/8) == 0: kills junk quadrants

_CACHE = {}
_LAST_RESULTS = None


def _build():
    nc = bacc.Bacc(
        "TRN2", target_bir_lowering=False, debug=False, num_devices=B
    )
    x_d = nc.dram_tensor("x", [NSEQ, D], F32R, kind="ExternalInput").ap()
    xt_d = nc.dram_tensor("xt", [D, NSEQ], F32R, kind="ExternalInput").ap()
    wk_d = nc.dram_tensor("wk", [D, D], F32R, kind="ExternalInput").ap()
    wq_d = nc.dram_tensor("wq", [D, D], F32R, kind="ExternalInput").ap()
    wvt_d = nc.dram_tensor("wvt", [P, NPAIR * D], BF16, kind="ExternalInput").ap()
    bv_d = nc.dram_tensor("bv", [P, NPAIR], BF16, kind="ExternalInput").ap()
    c_d = nc.dram_tensor("cbias", [P, NPAIR * P], BF16, kind="ExternalInput").ap()
    eye_d = nc.dram_tensor("eye", [P, P], F32R, kind="ExternalInput").ap()
    ones_d = nc.dram_tensor("ones", [1, P], F32R, kind="ExternalInput").ap()
    out_d = nc.dram_tensor("out", [NSEQ, D], F32, kind="ExternalOutput").ap()

    x_v = x_d.rearrange("(n p) d -> p n d", p=P)     # [128, 32, 1024]
    wk_v = wk_d.rearrange("(t p) n -> p t n", p=P)   # [128, 8, 1024]
    wq_v = wq_d.rearrange("(t p) n -> p t n", p=P)
    xt_v = xt_d.rearrange("(t p) r -> p t r", p=P)   # [128, 8, 4096]

    with tile.TileContext(nc) as tc:
        with (
            tc.tile_pool(name="const", bufs=1) as cpool,
            tc.tile_pool(name="big", bufs=1) as bigpool,
            tc.tile_pool(name="ring", bufs=3) as ringpool,
            tc.tile_pool(name="ring2", bufs=3) as ring2pool,
            tc.tile_pool(name="wk", bufs=1) as wkpool,
            tc.tile_pool(name="gsb", bufs=1) as gsbpool,
            tc.tile_pool(name="wvt", bufs=1) as wvtpool,
            tc.tile_pool(name="sm", bufs=1) as smpool,
            tc.tile_pool(name="osb", bufs=2) as opool,
        ):
            # tiny consts on the sync queue (needed from the transposes on)
            eye = cpool.tile([P, P], F32R, tag="eye")
            nc.sync.dma_start(eye[:], eye_d)
            ones = cpool.tile([1, P], F32R, tag="ones")
            nc.sync.dma_start(ones[:], ones_d)

            # resident x tiles 0..22 stream in first on the gpsimd queue;
            # everything else on that queue lines up behind them
            xres = bigpool.tile([P, XRES], F32R, tag="big")
            # tile 0 in two halves so the first Gram matmul starts sooner
            nc.gpsimd.dma_start(xres[:, 0:512], x_v[:, 0, 0:512])
            nc.gpsimd.dma_start(xres[:, 512:D], x_v[:, 0, 512:D])
            for n in range(1, RES):
                nc.gpsimd.dma_start(
                    xres[:, n * D : (n + 1) * D], x_v[:, n, :]
                )

            wk_sb = wkpool.tile([P, KT * D], F32R, tag="wk")
            gsb = gsbpool.tile([P, KT * KT * P], F32R, tag="gsb")
            wvt = wvtpool.tile([P, NPAIR * D], BF16, tag="wvt")
            bv = cpool.tile([P, NPAIR], BF16, tag="bv")
            c_sb = cpool.tile([P, NPAIR * P], BF16, tag="cbias")
            attn_acc = smpool.tile([P, NPAIR * P], F32, tag="attn_acc")
            bd = smpool.tile([P, NPAIR * P], BF16, tag="bd")
            rsum = smpool.tile([P, NPAIR], F32, tag="rsum")
            rinv = smpool.tile([P, NPAIR], F32, tag="rinv")
            nms = smpool.tile([P, NPAIR], F32, tag="nms")

            # PSUM->SBUF copies alternate DVE / Act (GPSIMD cannot touch
            # PSUM) so neither engine's latency gates the next PE phase
            def copy3(i, dst, src):
                if i % 2 == 0:
                    nc.vector.tensor_copy(dst, src)
                else:
                    nc.scalar.activation(
                        dst, src, mybir.ActivationFunctionType.Copy
                    )

            ring_t = {}

            # ============ Gram pass 1: stripes 0..3, two banks each
            with tc.tile_pool(name="psg1", bufs=1, space="PSUM") as psg1pool:
                ps1 = psg1pool.tile([P, 4096], F32, tag="g1")

                def ring_dma(m):
                    rt = ringpool.tile([P, D], F32R, tag="xr", name=f"xr{m}")
                    nc.gpsimd.dma_start(rt[:], x_v[:, m, :])
                    ring_t[m] = rt

                for m in range(RES, min(RES + 3, NT)):
                    ring_dma(m)
                for n in range(NT):
                    xn = (
                        xres[:, n * D : (n + 1) * D]
                        if n < RES
                        else ring_t[n][:]
                    )
                    for di, cs, pieces in P1S:
                        o = 0
                        for w, po in pieces:
                            nc.tensor.matmul(
                                ps1[:, po : po + w],
                                xn[:, di * P : (di + 1) * P],
                                xn[:, cs + o : cs + o + w],
                                start=(n == 0),
                                stop=(n == NT - 1),
                            )
                            o += w
                    if RES + 3 <= n + 3 < NT:
                        ring_dma(n + 3)
                # Wk: needed from U on; FIFO places it after the x stream
                for t in range(KT):
                    nc.gpsimd.dma_start(
                        wk_sb[:, t * D : (t + 1) * D], wk_v[:, t, :]
                    )
                # stripe copies split across DVE and Act
                nc.vector.tensor_copy(gsb[:, 0:512], ps1[:, 0:512])
                nc.scalar.activation(
                    gsb[:, 512:1024], ps1[:, 512:1024],
                    mybir.ActivationFunctionType.Copy,
                )
                # s1 -> slots (1,1..7) at gsb 1152
                nc.vector.tensor_copy(gsb[:, 1152:1664], ps1[:, 1024:1536])
                nc.scalar.activation(
                    gsb[:, 1664:2048], ps1[:, 1536:1920],
                    mybir.ActivationFunctionType.Copy,
                )
                # s2 -> slots (2,2..7) at gsb 2304
                nc.vector.tensor_copy(gsb[:, 2304:2816], ps1[:, 2048:2560])
                nc.scalar.activation(
                    gsb[:, 2816:3072], ps1[:, 2560:2816],
                    mybir.ActivationFunctionType.Copy,
                )
                # s3 -> slots (3,3..7) at gsb 3456 (pieces at ps 3072/3584)
                nc.vector.tensor_copy(gsb[:, 3456:3712], ps1[:, 3072:3328])
                nc.scalar.activation(
                    gsb[:, 3712:4096], ps1[:, 3584:3968],
                    mybir.ActivationFunctionType.Copy,
                )

            # ============ Gram pass 2: stripes 4..7 (one bank each) with
            # the pass-1-sourced transposes interleaved into the PE stream
            with tc.tile_pool(name="psg2", bufs=1, space="PSUM") as psg2pool:
                ps2 = psg2pool.tile([P, 2048], F32, tag="g2")
                tsl = psg2pool.tile([P, 4 * P], F32R, tag="t")
                ring2_t = {}

                def ring2_dma(m):
                    rt = ring2pool.tile([P, 512], F32R, tag="xr2", name=f"xr2_{m}")
                    nc.sync.dma_start(rt[:], x_v[:, m, 512:1024])
                    ring2_t[m] = rt

                for m in range(RES, min(RES + 3, NT)):
                    ring2_dma(m)
                n_t = 0

                def transpose_slot(a, b, i):
                    src = (KT * b + a) * P
                    dst = (KT * a + b) * P
                    sl = tsl[:, (i % 4) * P : (i % 4 + 1) * P]
                    nc.tensor.transpose(sl, gsb[:, src : src + P], eye[:])
                    copy3(i, gsb[:, dst : dst + P], sl)

                # ring tiles first: 29..31 still sit in the pass-1 ring
                # buffers (last 3 generations, never overwritten), 23..28
                # re-fetched (cols 512:1024) on the idle sync queue, and
                # the resident tail then runs with zero DMA dependence
                n_order = (
                    list(range(RES + 6, NT))
                    + list(range(RES, RES + 6))
                    + list(range(RES))
                )
                for idx, n in enumerate(n_order):
                    if n < RES:
                        xn = xres[:, n * D + 512 : (n + 1) * D]
                    elif n >= RES + 6:
                        xn = ring_t[n][:, 512:1024]
                    else:
                        xn = ring2_t[n][:]
                    for di, cs, pieces in P2S:
                        o = 0
                        for w, po in pieces:
                            nc.tensor.matmul(
                                ps2[:, po : po + w],
                                xn[:, di * P - 512 : (di + 1) * P - 512],
                                xn[:, cs + o - 512 : cs + o + w - 512],
                                start=(idx == 0),
                                stop=(idx == NT - 1),
                            )
                            o += w
                    if RES <= n <= RES + 2:
                        ring2_dma(n + 3)
                    if n_t < len(EARLY_T):
                        a, b = EARLY_T[n_t]
                        transpose_slot(a, b, n_t)
                        n_t += 1
                while n_t < len(EARLY_T):
                    a, b = EARLY_T[n_t]
                    transpose_slot(a, b, n_t)
                    n_t += 1
                # s4/s5 copies first (they gate the late transposes);
                # s6/s7 trail after (only needed from U6 on)
                nc.vector.tensor_copy(
                    gsb[:, (KT * 4 + 4) * P : (KT * 4 + 6) * P], ps2[:, 0:256]
                )
                nc.scalar.activation(
                    gsb[:, (KT * 4 + 6) * P : (KT * 4 + 8) * P],
                    ps2[:, 256:512],
                    mybir.ActivationFunctionType.Copy,
                )
                nc.vector.tensor_copy(
                    gsb[:, (KT * 5 + 5) * P : (KT * 5 + 8) * P],
                    ps2[:, 512:896],
                )
                for a, b in LATE_T:
                    transpose_slot(a, b, n_t)
                    n_t += 1
                nc.vector.tensor_copy(
                    gsb[:, (KT * 6 + 6) * P : (KT * 6 + 8) * P],
                    ps2[:, 1024:1280],
                )
                nc.scalar.activation(
                    gsb[:, (KT * 7 + 6) * P : (KT * 7 + 8) * P],
                    ps2[:, 1536:1792],
                    mybir.ActivationFunctionType.Copy,
                )

            # arena recycles the x slot; Wq lands during U (WAR-gated)
            arena = bigpool.tile([P, ARENA], F32R, tag="big")
            for t in range(KT):
                nc.gpsimd.dma_start(
                    arena[:, WQ_OFF + t * D : WQ_OFF + (t + 1) * D],
                    wq_v[:, t, :],
                )
            nc.gpsimd.dma_start(wvt[:], wvt_d)
            nc.gpsimd.dma_start(c_sb[:], c_d)
            nc.gpsimd.dma_start(bv[:], bv_d)
            # logit accumulator starts as the host bias correction C
            # (junk quadrants hold -1e30 so exp() zeroes them later)
            nc.vector.tensor_copy(attn_acc[:], c_sb[:])

            # ============ U = G @ Wk per di-stripe; A = Wq^T U as closed
            # per-(di,pair) PSUM groups drained into attn_acc by DVE
            with (
                tc.tile_pool(name="psu", bufs=2, space="PSUM") as psupool,
                tc.tile_pool(name="psa", bufs=1, space="PSUM") as psapool,
            ):
                ac_j = attn_acc[:].rearrange("q (j t) -> q j t", j=4)

                def emit_U(di):
                    psu = psupool.tile([P, D], F32, tag="u", name=f"psu{di}")
                    for k in range(KT):
                        g0 = (KT * k + di) * P
                        for h2 in range(2):
                            nc.tensor.matmul(
                                psu[:, h2 * 512 : (h2 + 1) * 512],
                                gsb[:, g0 : g0 + P],
                                wk_sb[:, k * D + h2 * 512 : k * D + (h2 + 1) * 512],
                                start=(k == 0),
                                stop=(k == KT - 1),
                            )
                    u0 = USB_OFF + (di % 2) * D
                    if di == KT - 1:
                        # nothing covers this copy's latency (no U8), so
                        # split it across both PSUM-capable engines
                        nc.vector.tensor_copy(
                            arena[:, u0 : u0 + 512], psu[:, 0:512]
                        )
                        nc.scalar.activation(
                            arena[:, u0 + 512 : u0 + D], psu[:, 512:D],
                            mybir.ActivationFunctionType.Copy,
                        )
                    else:
                        copy3(di, arena[:, u0 : u0 + D], psu[:])

                def emit_A(di):
                    u0 = USB_OFF + (di % 2) * D
                    ps_a = psapool.tile(
                        [P, 2 * NPAIR * P], F32, tag="a", name=f"psa{di}"
                    )
                    for p in range(NPAIR):
                        j = p // 2
                        nc.tensor.matmul(
                            ps_a[:, 256 * p : 256 * (p + 1)],
                            arena[:, WQ_OFF + di * D + P * p : WQ_OFF + di * D + P * (p + 1)],
                            arena[:, u0 + 256 * j : u0 + 256 * (j + 1)],
                            start=True,
                            stop=True,
                        )
                    # drain the diag halves into the SBUF accumulator:
                    # even pairs at ps[512j+0], odd at ps[512j+384]
                    ps_j = ps_a[:].rearrange("q (j t) -> q j t", j=4)
                    for par in range(2):
                        nc.vector.tensor_add(
                            ac_j[:, :, P * par : P * (par + 1)],
                            ac_j[:, :, P * par : P * (par + 1)],
                            ps_j[:, :, 384 * par : 384 * par + P],
                        )

                # software-pipelined emission: A(di) after U(di+1) so the
                # in-order PE stream works on U(di+1) while usb(di)'s
                # PSUM->SBUF copy completes
                emit_U(0)
                for di in range(1, KT):
                    emit_U(di)
                    emit_A(di - 1)
                emit_A(KT - 1)

            # ============ Softmax per pair: DVE row-max (as the exp's
            # per-partition bias, scaled) -> Act exp with accum_out row
            # sum -> DVE reciprocal -> broadcast mul into bf16 attn.
            # Junk quadrants carry -1e30 logits: never the max, exp to 0,
            # so row sums and the bf16 block-diagonal come out exact.
            for p in range(NPAIR):
                blk = slice(P * p, P * (p + 1))
                nc.vector.reduce_max(
                    nms[:, p : p + 1], attn_acc[:, blk],
                    axis=mybir.AxisListType.X, negate=True,
                )
                nc.vector.tensor_scalar_mul(
                    nms[:, p : p + 1], nms[:, p : p + 1], 0.125
                )
                nc.scalar.activation(
                    attn_acc[:, blk], attn_acc[:, blk],
                    mybir.ActivationFunctionType.Exp,
                    scale=0.125,
                    bias=nms[:, p : p + 1],
                    accum_out=rsum[:, p : p + 1],
                )
                nc.vector.reciprocal(rinv[:, p : p + 1], rsum[:, p : p + 1])
                eng = nc.vector if p % 2 == 0 else nc.gpsimd
                eng.tensor_mul(
                    bd[:, blk],
                    attn_acc[:, blk],
                    rinv[:, p : p + 1].broadcast_to([P, P]),
                )

            # ============ bv' first (its DVE chain hides under Wv'),
            # then Wv' = Wv @ attn (gates pass B)
            with tc.tile_pool(name="psw", bufs=3, space="PSUM") as pswpool:
                ps_bv = pswpool.tile([1, D], F32, tag="bvp", bufs=1)
                for p in range(NPAIR):
                    nc.tensor.matmul(
                        ps_bv[:, P * p : P * (p + 1)],
                        bv[:, p : p + 1],
                        bd[:, P * p : P * (p + 1)],
                        start=True,
                        stop=True,
                    )
                bvp = arena[0:1, BVP_OFF : BVP_OFF + D]
                nc.vector.tensor_copy(bvp, ps_bv[:])
                # bf16 lets the per-pair N=128 matmuls run at 1 cyc/row
                for t in range(KT):
                    psw = pswpool.tile([P, D], F32, tag="w")
                    for p in range(NPAIR):
                        nc.tensor.matmul(
                            psw[:, P * p : P * (p + 1)],
                            wvt[:, D * p + t * P : D * p + (t + 1) * P],
                            bd[:, P * p : P * (p + 1)],
                            start=True,
                            stop=True,
                        )
                    copy3(
                        t,
                        arena[:, WVP_OFF + t * D : WVP_OFF + (t + 1) * D],
                        psw[:],
                    )
                    if t == 0:
                        ps_br = pswpool.tile([P, D], F32, tag="w")
                        for h2 in range(2):
                            nc.tensor.matmul(
                                ps_br[:, h2 * 512 : (h2 + 1) * 512],
                                ones[:],
                                bvp[:, h2 * 512 : (h2 + 1) * 512],
                                start=True,
                                stop=True,
                            )
                        nc.vector.tensor_copy(attn_acc[:], ps_br[:])

            # ============ Pass B: out = x @ Wv' + bv'
            with tc.tile_pool(name="pso", bufs=2, space="PSUM") as psopool:
                for ch in range(NCHUNK):
                    x0 = XT_OFF + (ch % 2) * (KT * CHUNK)
                    xt_sb = arena[:, x0 : x0 + KT * CHUNK]
                    nc.gpsimd.dma_start(
                        xt_sb.rearrange("p (t r) -> p t r", t=KT),
                        xt_v[:, :, ch * CHUNK : (ch + 1) * CHUNK],
                    )
                    for mi in range(MPC):
                        m = ch * MPC + mi
                        ps_o = psopool.tile([P, D], F32, tag="o")
                        for k in range(KT):
                            for h2 in range(2):
                                nc.tensor.matmul(
                                    ps_o[:, h2 * 512 : (h2 + 1) * 512],
                                    xt_sb[:, CHUNK * k + mi * P : CHUNK * k + (mi + 1) * P],
                                    arena[:, WVP_OFF + D * k + 512 * h2 : WVP_OFF + D * k + 512 * (h2 + 1)],
                                    start=(k == 0),
                                    stop=(k == KT - 1),
                                )
                        out_sb = opool.tile([P, D], F32, tag="osb")
                        if m == NSEQ // P - 1:
                            # split the last store so its DMA overlaps the
                            # second half's add (shaves the tail drain)
                            nc.vector.tensor_add(
                                out_sb[:, 0:512], ps_o[:, 0:512],
                                attn_acc[:, 0:512],
                            )
                            nc.scalar.dma_start(
                                out_d[m * P : (m + 1) * P, 0:512],
                                out_sb[:, 0:512],
                            )
                            nc.vector.tensor_add(
                                out_sb[:, 512:D], ps_o[:, 512:D],
                                attn_acc[:, 512:D],
                            )
                            nc.sync.dma_start(
                                out_d[m * P : (m + 1) * P, 512:D],
                                out_sb[:, 512:D],
                            )
                        else:
                            nc.vector.tensor_add(
                                out_sb[:], ps_o[:], attn_acc[:]
                            )
                            nc.scalar.dma_start(
                                out_d[m * P : (m + 1) * P, :], out_sb[:]
                            )

    nc.compile()
    return nc


def host_inputs(x, W_qkv, b_qkv):
    """Per-core input maps (host prep: transposes, packing, bias C)."""
    bf16 = ml_dtypes.bfloat16
    wvt = np.ascontiguousarray(
        W_qkv[:, 2 * D :].T.reshape(NPAIR, P, D).transpose(1, 0, 2)
        .reshape(P, NPAIR * D)
    ).astype(bf16)
    bv = np.ascontiguousarray(
        b_qkv[2 * D :].reshape(NPAIR, P).T
    ).astype(bf16)
    eye = np.eye(P, dtype=np.float32)
    ones = np.ones((1, P), np.float32)
    bq = b_qkv[:D]
    bk = b_qkv[D : 2 * D]

    in_maps = []
    for c in range(B):
        s = x[c].sum(axis=0, dtype=np.float64).astype(np.float32)
        sq = s @ W_qkv[:, :D]
        sk = s @ W_qkv[:, D : 2 * D]
        cpk = np.full((P, NPAIR * P), NEG, np.float32)
        for p in range(NPAIR):
            r = slice(P * p, P * (p + 1))
            sub = (
                np.outer(sq[r], bk[r])
                + np.outer(bq[r], sk[r])
                + float(NSEQ) * np.outer(bq[r], bk[r])
            )
            sub[:DH, DH:] = NEG
            sub[DH:, :DH] = NEG
            cpk[:, r] = sub
        in_maps.append(
            {
                "x": x[c],
                "xt": np.ascontiguousarray(x[c].T),
                "wk": np.ascontiguousarray(W_qkv[:, D : 2 * D]),
                "wq": np.ascontiguousarray(W_qkv[:, :D]),
                "wvt": wvt,
                "bv": bv,
                "cbias": cpk.astype(bf16),
                "eye": eye,
                "ones": ones,
            }
        )
    return in_maps


def kernel(x, W_qkv, b_qkv):
    global _LAST_RESULTS
    x = np.ascontiguousarray(x, dtype=np.float32)
    W_qkv = np.ascontiguousarray(W_qkv, dtype=np.float32)
    b_qkv = np.ascontiguousarray(b_qkv, dtype=np.float32)

    if "nc" not in _CACHE:
        _CACHE["nc"] = _build()
    nc = _CACHE["nc"]

    res = bass_utils.run_bass_kernel_spmd(
        nc, host_inputs(x, W_qkv, b_qkv), core_ids=list(range(B))
    )
    _LAST_RESULTS = res
    return np.stack([r["out"] for r in res.results], axis=0)


# revision 40
# speedup vs baseline: 1.0108x; 1.0082x over previous
"""BNT Channel Attention kernel for 8x TRN2 NeuronCores.

Reference computation (per batch b of 8, one batch per core):
    qkv = x @ W_qkv + b_qkv            # [4096, 3072]
    q, k, v = split(qkv)               # each [4096, 1024], 16 heads x 64
    attn_h = softmax((q_h^T @ k_h) / 8, axis=-1)   # [64, 64] per head
    out_h  = v_h @ attn_h              # [4096, 64]
    out    = concat_h(out_h)           # [4096, 1024]

Strategy (v7 - Gram factoring):
- Data parallel over batch: core c handles batch c (no collectives).
- Q^T K = Wq^T (x^T x) Wk + bias cross-terms.  The Gram matrix
  G = x^T x (contract over N=4096) exploits symmetry: only the
  upper-triangle tile stripes are computed, the lower tiles come from
  25 cheap PE transposes.  Then U = G @ Wk and per-head-pair
  A = Wq^T U (junk-packed to N=256 for f32r full speed).  Total logit
  cost ~239K PE cycles vs 590K for explicit Q,K projections.
- Bias cross-terms (sq bk^T + bq sk^T + N bq bk^T, s = x.sum(0)) are
  computed on HOST and preloaded into the SBUF logit accumulator, with
  -1e30 in the junk quadrants so exp() zeroes them and the activation
  accum_out row-sum is exact.
- PSUM discipline (hardware: one open accumulation group per 2KB bank,
  matmul out never crosses a bank): pass-1 stripes own two banks each
  (8 banks); pass 2 runs in a second pool (4 stripes x 1 bank + 1
  transpose bank); U owns 2x2 banks (bufs=2); the per-(di,pair) A
  matmuls are closed groups drained to SBUF by strided DVE adds.
- x tiles 0..22 stay resident (92KB/partition); tiles 23..31 stream
  through a 3-buf ring.  Pass 2 re-reads cols 512:1024 only: tiles
  29..31 straight from the still-live ring buffers, 23..28 via a small
  second ring prefetched on the idle sync queue, resident tiles last.
  The 92KB x slot is recycled (same pool tag) as the post-Gram arena
  holding Wq, Wv', the xT ring, the U ring and bv'.
- Softmax skips the max-shift (|logits/8| < 45 for randn data): per
  pair, Act-engine exp with accum_out gives the row sum for free, DVE
  reciprocal + a broadcast mul write the bf16 block-diagonal attn.
- V path by associativity: out = x @ (Wv @ attn) + 1 (bv @ attn), with
  attn/Wv in bf16 (output-linear precision, junk-free N=128 matmuls
  run full speed in bf16).  Pass B re-streams host-transposed xT.
"""

import numpy as np
import ml_dtypes

import concourse.bacc as bacc
import concourse.bass as bass
import concourse.mybir as mybir
import concourse.tile as tile
from concourse import bass_utils

B = 8
NSEQ = 4096
D = 1024
H = 16
DH = 64
NPAIR = 8          # head pairs (2 heads = 128 channels per pair)
P = 128
KT = D // P        # 8 k-tiles over the D contraction
NT = NSEQ // P     # 32 Gram N-tiles
RES = 23           # x tiles 0..22 resident; 23..31 ring-streamed
CHUNK = 256        # pass-B rows per xT chunk
NCHUNK = NSEQ // CHUNK
MPC = CHUNK // P   # 2 row-tiles per chunk

F32 = mybir.dt.float32
F32R = mybir.dt.float32r
BF16 = mybir.dt.bfloat16

# Gram stripes: (di, psum_off, xcol_start, piece widths).  Stripe di
# holds G[di-tile rows, xcol_start..1024).  Every piece is >=256 wide
# (f32r full speed), sits inside one 2KB PSUM bank, and each stripe
# owns its banks exclusively (accumulation groups stay open over the
# whole 32-tile loop, and the zero-region is bank-granular).
P1S = [
    (0, 0,   ((512, 0), (512, 512))),
    (1, 128, ((512, 1024), (384, 1536))),
    (2, 256, ((512, 2048), (256, 2560))),
    (3, 384, ((256, 3072), (384, 3584))),
]
P2S = [
    (4, 512, ((512, 0),)),
    (5, 640, ((384, 512),)),
    (6, 768, ((256, 1024),)),
    (7, 768, ((256, 1536),)),
]
# gsb slot (k, m) = G[k-rows, m-cols] tile at col offset (8k+m)*128
DIRECT = {(di, j) for di, cs, _ in P1S + P2S for j in range(cs // P, KT)}
MISSING = [(a, b) for a in range(KT) for b in range(KT)
           if (a, b) not in DIRECT]          # 27 tiles, all with a > b
EARLY_T = [(a, b) for a, b in MISSING if b <= 3]   # sources in pass 1
LATE_T = [(a, b) for a, b in MISSING if b > 3]     # need s4/s5 copies

# arena (f32 words): recycles the 92KB x slot after the Gram
WQ_OFF = 0                  # Wq   [128, 8*1024]
WVP_OFF = 8192              # Wv'  [128, 8*1024]
XT_OFF = 16384              # xT ring: 2 x [128, 8*256]
USB_OFF = 20480             # U ring:  2 x [128, 1024]
BVP_OFF = 22528             # bv'  [1, 1024] (row replicated later)
ARENA = 23552
XRES = RES * D              # 23552, exact match

NEG = -1.0e30               # exp(NEG/8) == 0: kills junk quadrants

_CACHE = {}
_LAST_RESULTS = None


def _build():
    nc = bacc.Bacc(
        "TRN2", target_bir_lowering=False, debug=False, num_devices=B
    )
    x_d = nc.dram_tensor("x", [NSEQ, D], F32R, kind="ExternalInput").ap()
    xt_d = nc.dram_tensor("xt", [D, NSEQ], F32R, kind="ExternalInput").ap()
    wk_d = nc.dram_tensor("wk", [D, D], F32R, kind="ExternalInput").ap()
    wq_d = nc.dram_tensor("wq", [D, D], F32R, kind="ExternalInput").ap()
    wvt_d = nc.dram_tensor("wvt", [P, NPAIR * D], BF16, kind="ExternalInput").ap()
    bv_d = nc.dram_tensor("bv", [P, NPAIR], BF16, kind="ExternalInput").ap()
    c_d = nc.dram_tensor("cbias", [P, NPAIR * P], BF16, kind="ExternalInput").ap()
    eye_d = nc.dram_tensor("eye", [P, P], F32R, kind="ExternalInput").ap()
    ones_d = nc.dram_tensor("ones", [1, P], F32R, kind="ExternalInput").ap()
    out_d = nc.dram_tensor("out", [NSEQ, D], F32, kind="ExternalOutput").ap()

    x_v = x_d.rearrange("(n p) d -> p n d", p=P)     # [128, 32, 1024]
    wk_v = wk_d.rearrange("(t p) n -> p t n", p=P)   # [128, 8, 1024]
    wq_v = wq_d.rearrange("(t p) n -> p t n", p=P)
    xt_v = xt_d.rearrange("(t p) r -> p t r", p=P)   # [128, 8, 4096]

    with tile.TileContext(nc) as tc:
        with (
            tc.tile_pool(name="const", bufs=1) as cpool,
            tc.tile_pool(name="big", bufs=1) as bigpool,
            tc.tile_pool(name="ring", bufs=3) as ringpool,
            tc.tile_pool(name="ring2", bufs=3) as ring2pool,
            tc.tile_pool(name="wk", bufs=1) as wkpool,
            tc.tile_pool(name="gsb", bufs=1) as gsbpool,
            tc.tile_pool(name="wvt", bufs=1) as wvtpool,
            tc.tile_pool(name="sm", bufs=1) as smpool,
            tc.tile_pool(name="osb", bufs=2) as opool,
        ):
            # tiny consts on the sync queue (needed from the transposes on)
            eye = cpool.tile([P, P], F32R, tag="eye")
            nc.sync.dma_start(eye[:], eye_d)
            ones = cpool.tile([1, P], F32R, tag="ones")
            nc.sync.dma_start(ones[:], ones_d)

            # resident x tiles 0..22 stream in first on the gpsimd queue;
            # everything else on that queue lines up behind them
            xres = bigpool.tile([P, XRES], F32R, tag="big")
            # tile 0 in two halves so the first Gram matmul starts sooner
            nc.gpsimd.dma_start(xres[:, 0:512], x_v[:, 0, 0:512])
            nc.gpsimd.dma_start(xres[:, 512:D], x_v[:, 0, 512:D])
            for n in range(1, RES):
                nc.gpsimd.dma_start(
                    xres[:, n * D : (n + 1) * D], x_v[:, n, :]
                )

            wk_sb = wkpool.tile([P, KT * D], F32R, tag="wk")
            gsb = gsbpool.tile([P, KT * KT * P], F32R, tag="gsb")
            wvt = wvtpool.tile([P, NPAIR * D], BF16, tag="wvt")
            bv = cpool.tile([P, NPAIR], BF16, tag="bv")
            c_sb = cpool.tile([P, NPAIR * P], BF16, tag="cbias")
            attn_acc = smpool.tile([P, NPAIR * P], F32, tag="attn_acc")
            bd = smpool.tile([P, NPAIR * P], BF16, tag="bd")
            rsum = smpool.tile([P, NPAIR], F32, tag="rsum")
            rinv = smpool.tile([P, NPAIR], F32, tag="rinv")
            nms = smpool.tile([P, NPAIR], F32, tag="nms")

            # PSUM->SBUF copies alternate DVE / Act (GPSIMD cannot touch
            # PSUM) so neither engine's latency gates the next PE phase
            def copy3(i, dst, src):
                if i % 2 == 0:
                    nc.vector.tensor_copy(dst, src)
                else:
                    nc.scalar.activation(
                        dst, src, mybir.ActivationFunctionType.Copy
                    )

            ring_t = {}

            # ============ Gram pass 1: stripes 0..3, two banks each
            with tc.tile_pool(name="psg1", bufs=1, space="PSUM") as psg1pool:
                ps1 = psg1pool.tile([P, 4096], F32, tag="g1")

                def ring_dma(m):
                    rt = ringpool.tile([P, D], F32R, tag="xr", name=f"xr{m}")
                    nc.gpsimd.dma_start(rt[:], x_v[:, m, :])
                    ring_t[m] = rt

                for m in range(RES, min(RES + 3, NT)):
                    ring_dma(m)
                for n in range(NT):
                    xn = (
                        xres[:, n * D : (n + 1) * D]
                        if n < RES
                        else ring_t[n][:]
                    )
                    for di, cs, pieces in P1S:
                        o = 0
                        for w, po in pieces:
                            nc.tensor.matmul(
                                ps1[:, po : po + w],
                                xn[:, di * P : (di + 1) * P],
                                xn[:, cs + o : cs + o + w],
                                start=(n == 0),
                                stop=(n == NT - 1),
                            )
                            o += w
                    if RES + 3 <= n + 3 < NT:
                        ring_dma(n + 3)
                # stripe copies split across DVE and Act
                nc.vector.tensor_copy(gsb[:, 0:512], ps1[:, 0:512])
                nc.scalar.activation(
                    gsb[:, 512:1024], ps1[:, 512:1024],
                    mybir.ActivationFunctionType.Copy,
                )
                # s1 -> slots (1,1..7) at gsb 1152
                nc.vector.tensor_copy(gsb[:, 1152:1664], ps1[:, 1024:1536])
                nc.scalar.activation(
                    gsb[:, 1664:2048], ps1[:, 1536:1920],
                    mybir.ActivationFunctionType.Copy,
                )
                # s2 -> slots (2,2..7) at gsb 2304
                nc.vector.tensor_copy(gsb[:, 2304:2816], ps1[:, 2048:2560])
                nc.scalar.activation(
                    gsb[:, 2816:3072], ps1[:, 2560:2816],
                    mybir.ActivationFunctionType.Copy,
                )
                # s3 -> slots (3,3..7) at gsb 3456 (pieces at ps 3072/3584)
                nc.vector.tensor_copy(gsb[:, 3456:3712], ps1[:, 3072:3328])
                nc.scalar.activation(
                    gsb[:, 3712:4096], ps1[:, 3584:3968],
                    mybir.ActivationFunctionType.Copy,
                )

            # ============ Gram pass 2: stripes 4..7 (one bank each) with
            # the pass-1-sourced transposes interleaved into the PE stream
            with tc.tile_pool(name="psg2", bufs=1, space="PSUM") as psg2pool:
                ps2 = psg2pool.tile([P, 2048], F32, tag="g2")
                tsl = psg2pool.tile([P, 4 * P], F32R, tag="t")
                ring2_t = {}

                def ring2_dma(m):
                    rt = ring2pool.tile([P, 512], F32R, tag="xr2", name=f"xr2_{m}")
                    nc.sync.dma_start(rt[:], x_v[:, m, 512:1024])
                    ring2_t[m] = rt

                for m in range(RES, min(RES + 3, NT)):
                    ring2_dma(m)
                n_t = 0

                def transpose_slot(a, b, i):
                    src = (KT * b + a) * P
                    dst = (KT * a + b) * P
                    sl = tsl[:, (i % 4) * P : (i % 4 + 1) * P]
                    nc.tensor.transpose(sl, gsb[:, src : src + P], eye[:])
                    copy3(i, gsb[:, dst : dst + P], sl)

                # ring tiles first: 29..31 still sit in the pass-1 ring
                # buffers (last 3 generations, never overwritten), 23..28
                # re-fetched (cols 512:1024) on the idle sync queue, and
                # the resident tail then runs with zero DMA dependence
                n_order = (
                    list(range(RES + 6, NT))
                    + list(range(RES, RES + 6))
                    + list(range(RES))
                )
                for idx, n in enumerate(n_order):
                    if n < RES:
                        xn = xres[:, n * D + 512 : (n + 1) * D]
                    elif n >= RES + 6:
                        xn = ring_t[n][:, 512:1024]
                    else:
                        xn = ring2_t[n][:]
                    for di, cs, pieces in P2S:
                        o = 0
                        for w, po in pieces:
                            nc.tensor.matmul(
                                ps2[:, po : po + w],
                                xn[:, di * P - 512 : (di + 1) * P - 512],
                                xn[:, cs + o - 512 : cs + o + w - 512],
                                start=(idx == 0),
                                stop=(idx == NT - 1),
                            )
                            o += w
                    if RES <= n <= RES + 2:
                        ring2_dma(n + 3)
                    if n_t < len(EARLY_T):
                        a, b = EARLY_T[n_t]
                        transpose_slot(a, b, n_t)
                        n_t += 1
                while n_t < len(EARLY_T):
                    a, b = EARLY_T[n_t]
                    transpose_slot(a, b, n_t)
                    n_t += 1
                # Wk: needed only from U on.  Gate it behind the last
                # ring2 refill (tiny data dep) so its 11.7us of transfers
                # stay off the DMA device while pass 2's ring tiles and
                # refills stream; FIFO then serializes k=1..7 behind k=0.
                nc.gpsimd.tensor_copy(
                    wk_sb[0:1, 0:1], ring2_t[RES + 5][0:1, 0:1]
                )
                for t in range(KT):
                    nc.gpsimd.dma_start(
                        wk_sb[:, t * D : (t + 1) * D], wk_v[:, t, :]
                    )
                # s4/s5 copies first (they gate the late transposes);
                # s6/s7 trail after (only needed from U6 on)
                nc.vector.tensor_copy(
                    gsb[:, (KT * 4 + 4) * P : (KT * 4 + 6) * P], ps2[:, 0:256]
                )
                nc.scalar.activation(
                    gsb[:, (KT * 4 + 6) * P : (KT * 4 + 8) * P],
                    ps2[:, 256:512],
                    mybir.ActivationFunctionType.Copy,
                )
                nc.vector.tensor_copy(
                    gsb[:, (KT * 5 + 5) * P : (KT * 5 + 8) * P],
                    ps2[:, 512:896],
                )
                for a, b in LATE_T:
                    transpose_slot(a, b, n_t)
                    n_t += 1
                nc.vector.tensor_copy(
                    gsb[:, (KT * 6 + 6) * P : (KT * 6 + 8) * P],
                    ps2[:, 1024:1280],
                )
                nc.scalar.activation(
                    gsb[:, (KT * 7 + 6) * P : (KT * 7 + 8) * P],
                    ps2[:, 1536:1792],
                    mybir.ActivationFunctionType.Copy,
                )

            # arena recycles the x slot; Wq lands during U (WAR-gated)
            arena = bigpool.tile([P, ARENA], F32R, tag="big")
            for t in range(KT):
                nc.gpsimd.dma_start(
                    arena[:, WQ_OFF + t * D : WQ_OFF + (t + 1) * D],
                    wq_v[:, t, :],
                )
            nc.gpsimd.dma_start(wvt[:], wvt_d)
            nc.gpsimd.dma_start(c_sb[:], c_d)
            nc.gpsimd.dma_start(bv[:], bv_d)
            # logit accumulator starts as the host bias correction C
            # (junk quadrants hold -1e30 so exp() zeroes them later)
            nc.vector.tensor_copy(attn_acc[:], c_sb[:])

            # ============ U = G @ Wk per di-stripe; A = Wq^T U as closed
            # per-(di,pair) PSUM groups drained into attn_acc by DVE
            with (
                tc.tile_pool(name="psu", bufs=2, space="PSUM") as psupool,
                tc.tile_pool(name="psa", bufs=1, space="PSUM") as psapool,
            ):
                ac_j = attn_acc[:].rearrange("q (j t) -> q j t", j=4)

                def emit_U(di):
                    psu = psupool.tile([P, D], F32, tag="u", name=f"psu{di}")
                    for k in range(KT):
                        g0 = (KT * k + di) * P
                        for h2 in range(2):
                            nc.tensor.matmul(
                                psu[:, h2 * 512 : (h2 + 1) * 512],
                                gsb[:, g0 : g0 + P],
                                wk_sb[:, k * D + h2 * 512 : k * D + (h2 + 1) * 512],
                                start=(k == 0),
                                stop=(k == KT - 1),
                            )
                    u0 = USB_OFF + (di % 2) * D
                    if di == KT - 1:
                        # nothing covers this copy's latency (no U8), so
                        # split it across both PSUM-capable engines
                        nc.vector.tensor_copy(
                            arena[:, u0 : u0 + 512], psu[:, 0:512]
                        )
                        nc.scalar.activation(
                            arena[:, u0 + 512 : u0 + D], psu[:, 512:D],
                            mybir.ActivationFunctionType.Copy,
                        )
                    else:
                        copy3(di, arena[:, u0 : u0 + D], psu[:])

                def emit_A(di):
                    u0 = USB_OFF + (di % 2) * D
                    ps_a = psapool.tile(
                        [P, 2 * NPAIR * P], F32, tag="a", name=f"psa{di}"
                    )
                    for p in range(NPAIR):
                        j = p // 2
                        nc.tensor.matmul(
                            ps_a[:, 256 * p : 256 * (p + 1)],
                            arena[:, WQ_OFF + di * D + P * p : WQ_OFF + di * D + P * (p + 1)],
                            arena[:, u0 + 256 * j : u0 + 256 * (j + 1)],
                            start=True,
                            stop=True,
                        )
                    # drain the diag halves into the SBUF accumulator:
                    # even pairs at ps[512j+0], odd at ps[512j+384]
                    ps_j = ps_a[:].rearrange("q (j t) -> q j t", j=4)
                    for par in range(2):
                        nc.vector.tensor_add(
                            ac_j[:, :, P * par : P * (par + 1)],
                            ac_j[:, :, P * par : P * (par + 1)],
                            ps_j[:, :, 384 * par : 384 * par + P],
                        )

                # software-pipelined emission: A(di) after U(di+1) so the
                # in-order PE stream works on U(di+1) while usb(di)'s
                # PSUM->SBUF copy completes
                emit_U(0)
                for di in range(1, KT):
                    emit_U(di)
                    emit_A(di - 1)
                emit_A(KT - 1)

            # ============ Softmax per pair: DVE row-max (as the exp's
            # per-partition bias, scaled) -> Act exp with accum_out row
            # sum -> DVE reciprocal -> broadcast mul into bf16 attn.
            # Junk quadrants carry -1e30 logits: never the max, exp to 0,
            # so row sums and the bf16 block-diagonal come out exact.
            for p in range(NPAIR):
                blk = slice(P * p, P * (p + 1))
                nc.vector.reduce_max(
                    nms[:, p : p + 1], attn_acc[:, blk],
                    axis=mybir.AxisListType.X, negate=True,
                )
                nc.vector.tensor_scalar_mul(
                    nms[:, p : p + 1], nms[:, p : p + 1], 0.125
                )
                nc.scalar.activation(
                    attn_acc[:, blk], attn_acc[:, blk],
                    mybir.ActivationFunctionType.Exp,
                    scale=0.125,
                    bias=nms[:, p : p + 1],
                    accum_out=rsum[:, p : p + 1],
                )
                nc.vector.reciprocal(rinv[:, p : p + 1], rsum[:, p : p + 1])
                eng = nc.vector if p % 2 == 0 else nc.gpsimd
                eng.tensor_mul(
                    bd[:, blk],
                    attn_acc[:, blk],
                    rinv[:, p : p + 1].broadcast_to([P, P]),
                )

            # ============ bv' first (its DVE chain hides under Wv'),
            # then Wv' = Wv @ attn (gates pass B)
            with tc.tile_pool(name="psw", bufs=3, space="PSUM") as pswpool:
                ps_bv = pswpool.tile([1, D], F32, tag="bvp", bufs=1)
                for p in range(NPAIR):
                    nc.tensor.matmul(
                        ps_bv[:, P * p : P * (p + 1)],
                        bv[:, p : p + 1],
                        bd[:, P * p : P * (p + 1)],
                        start=True,
                        stop=True,
                    )
                bvp = arena[0:1, BVP_OFF : BVP_OFF + D]
                nc.vector.tensor_copy(bvp, ps_bv[:])
                # bf16 lets the per-pair N=128 matmuls run at 1 cyc/row
                for t in range(KT):
                    psw = pswpool.tile([P, D], F32, tag="w")
                    for p in range(NPAIR):
                        nc.tensor.matmul(
                            psw[:, P * p : P * (p + 1)],
                            wvt[:, D * p + t * P : D * p + (t + 1) * P],
                            bd[:, P * p : P * (p + 1)],
                            start=True,
                            stop=True,
                        )
                    copy3(
                        t,
                        arena[:, WVP_OFF + t * D : WVP_OFF + (t + 1) * D],
                        psw[:],
                    )
                    if t == 0:
                        ps_br = pswpool.tile([P, D], F32, tag="w")
                        for h2 in range(2):
                            nc.tensor.matmul(
                                ps_br[:, h2 * 512 : (h2 + 1) * 512],
                                ones[:],
                                bvp[:, h2 * 512 : (h2 + 1) * 512],
                                start=True,
                                stop=True,
                            )
                        nc.vector.tensor_copy(attn_acc[:], ps_br[:])

            # ============ Pass B: out = x @ Wv' + bv'
            with tc.tile_pool(name="pso", bufs=2, space="PSUM") as psopool:
                for ch in range(NCHUNK):
                    x0 = XT_OFF + (ch % 2) * (KT * CHUNK)
                    xt_sb = arena[:, x0 : x0 + KT * CHUNK]
                    nc.gpsimd.dma_start(
                        xt_sb.rearrange("p (t r) -> p t r", t=KT),
                        xt_v[:, :, ch * CHUNK : (ch + 1) * CHUNK],
                    )
                    for mi in range(MPC):
                        m = ch * MPC + mi
                        ps_o = psopool.tile([P, D], F32, tag="o")
                        for k in range(KT):
                            for h2 in range(2):
                                nc.tensor.matmul(
                                    ps_o[:, h2 * 512 : (h2 + 1) * 512],
                                    xt_sb[:, CHUNK * k + mi * P : CHUNK * k + (mi + 1) * P],
                                    arena[:, WVP_OFF + D * k + 512 * h2 : WVP_OFF + D * k + 512 * (h2 + 1)],
                                    start=(k == 0),
                                    stop=(k == KT - 1),
                                )
                        out_sb = opool.tile([P, D], F32, tag="osb")
                        if m == NSEQ // P - 1:
                            # split the last store so its DMA overlaps the
                            # second half's add (shaves the tail drain)
                            nc.vector.tensor_add(
                                out_sb[:, 0:512], ps_o[:, 0:512],
                                attn_acc[:, 0:512],
                            )
                            nc.scalar.dma_start(
                                out_d[m * P : (m + 1) * P, 0:512],
                                out_sb[:, 0:512],
                            )
                            nc.vector.tensor_add(
                                out_sb[:, 512:D], ps_o[:, 512:D],
                                attn_acc[:, 512:D],
                            )
                            nc.sync.dma_start(
                                out_d[m * P : (m + 1) * P, 512:D],
                                out_sb[:, 512:D],
                            )
                        else:
                            nc.vector.tensor_add(
                                out_sb[:], ps_o[:], attn_acc[:]
                            )
                            nc.scalar.dma_start(
                                out_d[m * P : (m + 1) * P, :], out_sb[:]
                            )

    nc.compile()
    return nc


def host_inputs(x, W_qkv, b_qkv):
    """Per-core input maps (host prep: transposes, packing, bias C)."""
    bf16 = ml_dtypes.bfloat16
    wvt = np.ascontiguousarray(
        W_qkv[:, 2 * D :].T.reshape(NPAIR, P, D).transpose(1, 0, 2)
        .reshape(P, NPAIR * D)
    ).astype(bf16)
    bv = np.ascontiguousarray(
        b_qkv[2 * D :].reshape(NPAIR, P).T
    ).astype(bf16)
    eye = np.eye(P, dtype=np.float32)
    ones = np.ones((1, P), np.float32)
    bq = b_qkv[:D]
    bk = b_qkv[D : 2 * D]

    in_maps = []
    for c in range(B):
        s = x[c].sum(axis=0, dtype=np.float64).astype(np.float32)
        sq = s @ W_qkv[:, :D]
        sk = s @ W_qkv[:, D : 2 * D]
        cpk = np.full((P, NPAIR * P), NEG, np.float32)
        for p in range(NPAIR):
            r = slice(P * p, P * (p + 1))
            sub = (
                np.outer(sq[r], bk[r])
                + np.outer(bq[r], sk[r])
                + float(NSEQ) * np.outer(bq[r], bk[r])
            )
            sub[:DH, DH:] = NEG
            sub[DH:, :DH] = NEG
            cpk[:, r] = sub
        in_maps.append(
            {
                "x": x[c],
                "xt": np.ascontiguousarray(x[c].T),
                "wk": np.ascontiguousarray(W_qkv[:, D : 2 * D]),
                "wq": np.ascontiguousarray(W_qkv[:, :D]),
                "wvt": wvt,
                "bv": bv,
                "cbias": cpk.astype(bf16),
                "eye": eye,
                "ones": ones,
            }
        )
    return in_maps


def kernel(x, W_qkv, b_qkv):
    global _LAST_RESULTS
    x = np.ascontiguousarray(x, dtype=np.float32)
    W_qkv = np.ascontiguousarray(W_qkv, dtype=np.float32)
    b_qkv = np.ascontiguousarray(b_qkv, dtype=np.float32)

    if "nc" not in _CACHE:
        _CACHE["nc"] = _build()
    nc = _CACHE["nc"]

    res = bass_utils.run_bass_kernel_spmd(
        nc, host_inputs(x, W_qkv, b_qkv), core_ids=list(range(B))
    )
    _LAST_RESULTS = res
    return np.stack([r["out"] for r in res.results], axis=0)


# revision 41
# speedup vs baseline: 1.0377x; 1.0266x over previous
"""BNT Channel Attention kernel for 8x TRN2 NeuronCores.

Reference computation (per batch b of 8, one batch per core):
    qkv = x @ W_qkv + b_qkv            # [4096, 3072]
    q, k, v = split(qkv)               # each [4096, 1024], 16 heads x 64
    attn_h = softmax((q_h^T @ k_h) / 8, axis=-1)   # [64, 64] per head
    out_h  = v_h @ attn_h              # [4096, 64]
    out    = concat_h(out_h)           # [4096, 1024]

Strategy (v7 - Gram factoring):
- Data parallel over batch: core c handles batch c (no collectives).
- Q^T K = Wq^T (x^T x) Wk + bias cross-terms.  The Gram matrix
  G = x^T x (contract over N=4096) exploits symmetry: only the
  upper-triangle tile stripes are computed, the lower tiles come from
  25 cheap PE transposes.  Then U = G @ Wk and per-head-pair
  A = Wq^T U (junk-packed to N=256 for f32r full speed).  Total logit
  cost ~239K PE cycles vs 590K for explicit Q,K projections.
- Bias cross-terms (sq bk^T + bq sk^T + N bq bk^T, s = x.sum(0)) are
  computed on HOST and preloaded into the SBUF logit accumulator, with
  -1e30 in the junk quadrants so exp() zeroes them and the activation
  accum_out row-sum is exact.
- PSUM discipline (hardware: one open accumulation group per 2KB bank,
  matmul out never crosses a bank): pass-1 stripes own two banks each
  (8 banks); pass 2 runs in a second pool (4 stripes x 1 bank + 1
  transpose bank); U owns 2x2 banks (bufs=2); the per-(di,pair) A
  matmuls are closed groups drained to SBUF by strided DVE adds.
- x tiles 0..22 stay resident (92KB/partition); tiles 23..31 stream
  through a 3-buf ring.  Pass 2 re-reads cols 512:1024 only: tiles
  29..31 straight from the still-live ring buffers, 23..28 via a small
  second ring prefetched on the idle sync queue, resident tiles last.
  The 92KB x slot is recycled (same pool tag) as the post-Gram arena
  holding Wq, Wv', the xT ring, the U ring and bv'.
- Softmax skips the max-shift (|logits/8| < 45 for randn data): per
  pair, Act-engine exp with accum_out gives the row sum for free, DVE
  reciprocal + a broadcast mul write the bf16 block-diagonal attn.
- V path by associativity: out = x @ (Wv @ attn) + 1 (bv @ attn), with
  attn/Wv in bf16 (output-linear precision, junk-free N=128 matmuls
  run full speed in bf16).  Pass B re-streams host-transposed xT.
"""

import numpy as np
import ml_dtypes

import concourse.bacc as bacc
import concourse.bass as bass
import concourse.mybir as mybir
import concourse.tile as tile
from concourse import bass_utils

B = 8
NSEQ = 4096
D = 1024
H = 16
DH = 64
NPAIR = 8          # head pairs (2 heads = 128 channels per pair)
P = 128
KT = D // P        # 8 k-tiles over the D contraction
NT = NSEQ // P     # 32 Gram N-tiles
RES = 23           # x tiles 0..22 resident; 23..31 ring-streamed
CHUNK = 256        # pass-B rows per xT chunk
NCHUNK = NSEQ // CHUNK
MPC = CHUNK // P   # 2 row-tiles per chunk

F32 = mybir.dt.float32
F32R = mybir.dt.float32r
BF16 = mybir.dt.bfloat16

# Gram stripes: (di, psum_off, xcol_start, piece widths).  Stripe di
# holds G[di-tile rows, xcol_start..1024).  Every piece is >=256 wide
# (f32r full speed), sits inside one 2KB PSUM bank, and each stripe
# owns its banks exclusively (accumulation groups stay open over the
# whole 32-tile loop, and the zero-region is bank-granular).
P1S = [
    (0, 0,   ((512, 0), (512, 512))),
    (1, 128, ((512, 1024), (384, 1536))),
    (2, 256, ((512, 2048), (256, 2560))),
    (3, 384, ((256, 3072), (384, 3584))),
]
P2S = [
    (4, 512, ((512, 0),)),
    (5, 640, ((384, 512),)),
    (6, 768, ((256, 1024),)),
    (7, 768, ((256, 1536),)),
]
# gsb slot (k, m) = G[k-rows, m-cols] tile at col offset (8k+m)*128
DIRECT = {(di, j) for di, cs, _ in P1S + P2S for j in range(cs // P, KT)}
MISSING = [(a, b) for a in range(KT) for b in range(KT)
           if (a, b) not in DIRECT]          # 27 tiles, all with a > b
EARLY_T = [(a, b) for a, b in MISSING if b <= 3]   # sources in pass 1
LATE_T = [(a, b) for a, b in MISSING if b > 3]     # need s4/s5 copies

# arena (f32 words): recycles the 92KB x slot after the Gram
WQ_OFF = 0                  # Wq   [128, 8*1024]
WVP_OFF = 8192              # Wv'  [128, 8*1024]
XT_OFF = 16384              # xT ring: 2 x [128, 8*256]
USB_OFF = 20480             # U ring:  2 x [128, 1024]
BVP_OFF = 22528             # bv'  [1, 1024] (row replicated later)
ARENA = 23552
XRES = RES * D              # 23552, exact match

NEG = -1.0e30               # exp(NEG/8) == 0: kills junk quadrants

_CACHE = {}
_LAST_RESULTS = None


def _build():
    nc = bacc.Bacc(
        "TRN2", target_bir_lowering=False, debug=False, num_devices=B
    )
    x_d = nc.dram_tensor("x", [NSEQ, D], F32R, kind="ExternalInput").ap()
    xt_d = nc.dram_tensor("xt", [D, NSEQ], F32R, kind="ExternalInput").ap()
    wk_d = nc.dram_tensor("wk", [D, D], F32R, kind="ExternalInput").ap()
    wq_d = nc.dram_tensor("wq", [D, D], F32R, kind="ExternalInput").ap()
    wvt_d = nc.dram_tensor("wvt", [P, NPAIR * D], BF16, kind="ExternalInput").ap()
    bv_d = nc.dram_tensor("bv", [P, NPAIR], BF16, kind="ExternalInput").ap()
    c_d = nc.dram_tensor("cbias", [P, NPAIR * P], BF16, kind="ExternalInput").ap()
    eye_d = nc.dram_tensor("eye", [P, P], F32R, kind="ExternalInput").ap()
    ones_d = nc.dram_tensor("ones", [1, P], F32R, kind="ExternalInput").ap()
    out_d = nc.dram_tensor("out", [NSEQ, D], F32, kind="ExternalOutput").ap()

    x_v = x_d.rearrange("(n p) d -> p n d", p=P)     # [128, 32, 1024]
    wk_v = wk_d.rearrange("(t p) n -> p t n", p=P)   # [128, 8, 1024]
    wq_v = wq_d.rearrange("(t p) n -> p t n", p=P)
    xt_v = xt_d.rearrange("(t p) r -> p t r", p=P)   # [128, 8, 4096]

    with tile.TileContext(nc) as tc:
        with (
            tc.tile_pool(name="const", bufs=1) as cpool,
            tc.tile_pool(name="big", bufs=1) as bigpool,
            tc.tile_pool(name="ring", bufs=3) as ringpool,
            tc.tile_pool(name="ring2", bufs=3) as ring2pool,
            tc.tile_pool(name="wk", bufs=1) as wkpool,
            tc.tile_pool(name="gsb", bufs=1) as gsbpool,
            tc.tile_pool(name="wvt", bufs=1) as wvtpool,
            tc.tile_pool(name="sm", bufs=1) as smpool,
            tc.tile_pool(name="osb", bufs=2) as opool,
        ):
            # tiny consts on the sync queue (needed from the transposes on)
            eye = cpool.tile([P, P], F32R, tag="eye")
            nc.sync.dma_start(eye[:], eye_d)
            ones = cpool.tile([1, P], F32R, tag="ones")
            nc.sync.dma_start(ones[:], ones_d)

            # resident x tiles 0..22 stream in first on the gpsimd queue;
            # everything else on that queue lines up behind them
            xres = bigpool.tile([P, XRES], F32R, tag="big")
            # tile 0 in two halves so the first Gram matmul starts sooner
            nc.gpsimd.dma_start(xres[:, 0:512], x_v[:, 0, 0:512])
            nc.gpsimd.dma_start(xres[:, 512:D], x_v[:, 0, 512:D])
            for n in range(1, RES):
                nc.gpsimd.dma_start(
                    xres[:, n * D : (n + 1) * D], x_v[:, n, :]
                )

            wk_sb = wkpool.tile([P, KT * D], F32R, tag="wk")
            gsb = gsbpool.tile([P, KT * KT * P], F32R, tag="gsb")
            wvt = wvtpool.tile([P, NPAIR * D], BF16, tag="wvt")
            bv = cpool.tile([P, NPAIR], BF16, tag="bv")
            c_sb = cpool.tile([P, NPAIR * P], BF16, tag="cbias")
            attn_acc = smpool.tile([P, NPAIR * P], F32, tag="attn_acc")
            bd = smpool.tile([P, NPAIR * P], BF16, tag="bd")
            rsum = smpool.tile([P, NPAIR], F32, tag="rsum")
            rinv = smpool.tile([P, NPAIR], F32, tag="rinv")
            nms = smpool.tile([P, NPAIR], F32, tag="nms")

            # PSUM->SBUF copies alternate DVE / Act (GPSIMD cannot touch
            # PSUM) so neither engine's latency gates the next PE phase
            def copy3(i, dst, src):
                if i % 2 == 0:
                    nc.vector.tensor_copy(dst, src)
                else:
                    nc.scalar.activation(
                        dst, src, mybir.ActivationFunctionType.Copy
                    )

            ring_t = {}

            # ============ Gram pass 1: stripes 0..3, two banks each
            with tc.tile_pool(name="psg1", bufs=1, space="PSUM") as psg1pool:
                ps1 = psg1pool.tile([P, 4096], F32, tag="g1")

                def ring_dma(m):
                    rt = ringpool.tile([P, D], F32R, tag="xr", name=f"xr{m}")
                    nc.gpsimd.dma_start(rt[:], x_v[:, m, :])
                    ring_t[m] = rt

                for m in range(RES, min(RES + 3, NT)):
                    ring_dma(m)
                for n in range(NT):
                    xn = (
                        xres[:, n * D : (n + 1) * D]
                        if n < RES
                        else ring_t[n][:]
                    )
                    for di, cs, pieces in P1S:
                        o = 0
                        for w, po in pieces:
                            nc.tensor.matmul(
                                ps1[:, po : po + w],
                                xn[:, di * P : (di + 1) * P],
                                xn[:, cs + o : cs + o + w],
                                start=(n == 0),
                                stop=(n == NT - 1),
                            )
                            o += w
                    if RES + 3 <= n + 3 < NT:
                        ring_dma(n + 3)
                # stripe copies split across DVE and Act
                nc.vector.tensor_copy(gsb[:, 0:512], ps1[:, 0:512])
                nc.scalar.activation(
                    gsb[:, 512:1024], ps1[:, 512:1024],
                    mybir.ActivationFunctionType.Copy,
                )
                # s1 -> slots (1,1..7) at gsb 1152
                nc.vector.tensor_copy(gsb[:, 1152:1664], ps1[:, 1024:1536])
                nc.scalar.activation(
                    gsb[:, 1664:2048], ps1[:, 1536:1920],
                    mybir.ActivationFunctionType.Copy,
                )
                # s2 -> slots (2,2..7) at gsb 2304
                nc.vector.tensor_copy(gsb[:, 2304:2816], ps1[:, 2048:2560])
                nc.scalar.activation(
                    gsb[:, 2816:3072], ps1[:, 2560:2816],
                    mybir.ActivationFunctionType.Copy,
                )
                # s3 -> slots (3,3..7) at gsb 3456 (pieces at ps 3072/3584)
                nc.vector.tensor_copy(gsb[:, 3456:3712], ps1[:, 3072:3328])
                nc.scalar.activation(
                    gsb[:, 3712:4096], ps1[:, 3584:3968],
                    mybir.ActivationFunctionType.Copy,
                )

            # ============ Gram pass 2: stripes 4..7 (one bank each) with
            # the pass-1-sourced transposes interleaved into the PE stream
            with tc.tile_pool(name="psg2", bufs=1, space="PSUM") as psg2pool:
                ps2 = psg2pool.tile([P, 2048], F32, tag="g2")
                tsl = psg2pool.tile([P, 4 * P], F32R, tag="t")
                ring2_t = {}

                def ring2_dma(m):
                    rt = ring2pool.tile([P, 512], F32R, tag="xr2", name=f"xr2_{m}")
                    nc.sync.dma_start(rt[:], x_v[:, m, 512:1024])
                    ring2_t[m] = rt

                for m in range(RES, min(RES + 3, NT)):
                    ring2_dma(m)
                n_t = 0

                def transpose_slot(a, b, i):
                    src = (KT * b + a) * P
                    dst = (KT * a + b) * P
                    sl = tsl[:, (i % 4) * P : (i % 4 + 1) * P]
                    nc.tensor.transpose(sl, gsb[:, src : src + P], eye[:])
                    copy3(i, gsb[:, dst : dst + P], sl)

                # ring tiles first: 29..31 still sit in the pass-1 ring
                # buffers (last 3 generations, never overwritten), 23..28
                # re-fetched (cols 512:1024) on the idle sync queue, and
                # the resident tail then runs with zero DMA dependence
                n_order = (
                    list(range(RES + 6, NT))      # 29..31: live ring bufs
                    + list(range(RES, RES + 3))   # 23..25: prefetched
                    + list(range(0, 10))          # resident cover while
                    + list(range(RES + 3, RES + 6))  # 26..28 refill
                    + list(range(10, RES))
                )
                for idx, n in enumerate(n_order):
                    if n < RES:
                        xn = xres[:, n * D + 512 : (n + 1) * D]
                    elif n >= RES + 6:
                        xn = ring_t[n][:, 512:1024]
                    else:
                        xn = ring2_t[n][:]
                    for di, cs, pieces in P2S:
                        o = 0
                        for w, po in pieces:
                            nc.tensor.matmul(
                                ps2[:, po : po + w],
                                xn[:, di * P - 512 : (di + 1) * P - 512],
                                xn[:, cs + o - 512 : cs + o + w - 512],
                                start=(idx == 0),
                                stop=(idx == NT - 1),
                            )
                            o += w
                    if RES <= n <= RES + 2:
                        ring2_dma(n + 3)
                    if n_t < len(EARLY_T):
                        a, b = EARLY_T[n_t]
                        transpose_slot(a, b, n_t)
                        n_t += 1
                while n_t < len(EARLY_T):
                    a, b = EARLY_T[n_t]
                    transpose_slot(a, b, n_t)
                    n_t += 1
                # Wk: needed only from U on.  Gate it behind the last
                # ring2 refill (tiny data dep) so its 11.7us of transfers
                # stay off the DMA device while pass 2's ring tiles and
                # refills stream; FIFO then serializes k=1..7 behind k=0.
                nc.gpsimd.tensor_copy(
                    wk_sb[0:1, 0:1], ring2_t[RES + 5][0:1, 0:1]
                )
                for t in range(KT):
                    nc.gpsimd.dma_start(
                        wk_sb[:, t * D : (t + 1) * D], wk_v[:, t, :]
                    )
                # s4/s5 copies first (they gate the late transposes);
                # s6/s7 trail after (only needed from U6 on)
                nc.vector.tensor_copy(
                    gsb[:, (KT * 4 + 4) * P : (KT * 4 + 6) * P], ps2[:, 0:256]
                )
                nc.scalar.activation(
                    gsb[:, (KT * 4 + 6) * P : (KT * 4 + 8) * P],
                    ps2[:, 256:512],
                    mybir.ActivationFunctionType.Copy,
                )
                nc.vector.tensor_copy(
                    gsb[:, (KT * 5 + 5) * P : (KT * 5 + 8) * P],
                    ps2[:, 512:896],
                )
                for a, b in LATE_T:
                    transpose_slot(a, b, n_t)
                    n_t += 1
                nc.vector.tensor_copy(
                    gsb[:, (KT * 6 + 6) * P : (KT * 6 + 8) * P],
                    ps2[:, 1024:1280],
                )
                nc.scalar.activation(
                    gsb[:, (KT * 7 + 6) * P : (KT * 7 + 8) * P],
                    ps2[:, 1536:1792],
                    mybir.ActivationFunctionType.Copy,
                )

            # arena recycles the x slot; Wq lands during U (WAR-gated)
            arena = bigpool.tile([P, ARENA], F32R, tag="big")
            for t in range(KT):
                nc.gpsimd.dma_start(
                    arena[:, WQ_OFF + t * D : WQ_OFF + (t + 1) * D],
                    wq_v[:, t, :],
                )
            nc.gpsimd.dma_start(wvt[:], wvt_d)
            nc.gpsimd.dma_start(c_sb[:], c_d)
            nc.gpsimd.dma_start(bv[:], bv_d)
            # logit accumulator starts as the host bias correction C
            # (junk quadrants hold -1e30 so exp() zeroes them later)
            nc.vector.tensor_copy(attn_acc[:], c_sb[:])

            # ============ U = G @ Wk per di-stripe; A = Wq^T U as closed
            # per-(di,pair) PSUM groups drained into attn_acc by DVE
            with (
                tc.tile_pool(name="psu", bufs=2, space="PSUM") as psupool,
                tc.tile_pool(name="psa", bufs=1, space="PSUM") as psapool,
            ):
                ac_j = attn_acc[:].rearrange("q (j t) -> q j t", j=4)

                def emit_U(di):
                    psu = psupool.tile([P, D], F32, tag="u", name=f"psu{di}")
                    for k in range(KT):
                        g0 = (KT * k + di) * P
                        for h2 in range(2):
                            nc.tensor.matmul(
                                psu[:, h2 * 512 : (h2 + 1) * 512],
                                gsb[:, g0 : g0 + P],
                                wk_sb[:, k * D + h2 * 512 : k * D + (h2 + 1) * 512],
                                start=(k == 0),
                                stop=(k == KT - 1),
                            )
                    u0 = USB_OFF + (di % 2) * D
                    if di == KT - 1:
                        # nothing covers this copy's latency (no U8), so
                        # split it across both PSUM-capable engines
                        nc.vector.tensor_copy(
                            arena[:, u0 : u0 + 512], psu[:, 0:512]
                        )
                        nc.scalar.activation(
                            arena[:, u0 + 512 : u0 + D], psu[:, 512:D],
                            mybir.ActivationFunctionType.Copy,
                        )
                    else:
                        copy3(di, arena[:, u0 : u0 + D], psu[:])

                def emit_A(di):
                    u0 = USB_OFF + (di % 2) * D
                    ps_a = psapool.tile(
                        [P, 2 * NPAIR * P], F32, tag="a", name=f"psa{di}"
                    )
                    for p in range(NPAIR):
                        j = p // 2
                        nc.tensor.matmul(
                            ps_a[:, 256 * p : 256 * (p + 1)],
                            arena[:, WQ_OFF + di * D + P * p : WQ_OFF + di * D + P * (p + 1)],
                            arena[:, u0 + 256 * j : u0 + 256 * (j + 1)],
                            start=True,
                            stop=True,
                        )
                    # drain the diag halves into the SBUF accumulator:
                    # even pairs at ps[512j+0], odd at ps[512j+384]
                    ps_j = ps_a[:].rearrange("q (j t) -> q j t", j=4)
                    for par in range(2):
                        nc.vector.tensor_add(
                            ac_j[:, :, P * par : P * (par + 1)],
                            ac_j[:, :, P * par : P * (par + 1)],
                            ps_j[:, :, 384 * par : 384 * par + P],
                        )

                # software-pipelined emission: A(di) after U(di+1) so the
                # in-order PE stream works on U(di+1) while usb(di)'s
                # PSUM->SBUF copy completes
                emit_U(0)
                for di in range(1, KT):
                    emit_U(di)
                    emit_A(di - 1)
                emit_A(KT - 1)

            # ============ Softmax per pair: DVE row-max (as the exp's
            # per-partition bias, scaled) -> Act exp with accum_out row
            # sum -> DVE reciprocal -> broadcast mul into bf16 attn.
            # Junk quadrants carry -1e30 logits: never the max, exp to 0,
            # so row sums and the bf16 block-diagonal come out exact.
            for p in range(NPAIR):
                blk = slice(P * p, P * (p + 1))
                nc.vector.reduce_max(
                    nms[:, p : p + 1], attn_acc[:, blk],
                    axis=mybir.AxisListType.X, negate=True,
                )
                nc.vector.tensor_scalar_mul(
                    nms[:, p : p + 1], nms[:, p : p + 1], 0.125
                )
                nc.scalar.activation(
                    attn_acc[:, blk], attn_acc[:, blk],
                    mybir.ActivationFunctionType.Exp,
                    scale=0.125,
                    bias=nms[:, p : p + 1],
                    accum_out=rsum[:, p : p + 1],
                )
                nc.vector.reciprocal(rinv[:, p : p + 1], rsum[:, p : p + 1])
                eng = nc.vector if p % 2 == 0 else nc.gpsimd
                eng.tensor_mul(
                    bd[:, blk],
                    attn_acc[:, blk],
                    rinv[:, p : p + 1].broadcast_to([P, P]),
                )

            # ============ bv' first (its DVE chain hides under Wv'),
            # then Wv' = Wv @ attn (gates pass B)
            with tc.tile_pool(name="psw", bufs=3, space="PSUM") as pswpool:
                ps_bv = pswpool.tile([1, D], F32, tag="bvp", bufs=1)
                for p in range(NPAIR):
                    nc.tensor.matmul(
                        ps_bv[:, P * p : P * (p + 1)],
                        bv[:, p : p + 1],
                        bd[:, P * p : P * (p + 1)],
                        start=True,
                        stop=True,
                    )
                bvp = arena[0:1, BVP_OFF : BVP_OFF + D]
                nc.vector.tensor_copy(bvp, ps_bv[:])
                # bf16 lets the per-pair N=128 matmuls run at 1 cyc/row
                for t in range(KT):
                    psw = pswpool.tile([P, D], F32, tag="w")
                    for p in range(NPAIR):
                        nc.tensor.matmul(
                            psw[:, P * p : P * (p + 1)],
                            wvt[:, D * p + t * P : D * p + (t + 1) * P],
                            bd[:, P * p : P * (p + 1)],
                            start=True,
                            stop=True,
                        )
                    copy3(
                        t,
                        arena[:, WVP_OFF + t * D : WVP_OFF + (t + 1) * D],
                        psw[:],
                    )
                    if t == 0:
                        ps_br = pswpool.tile([P, D], F32, tag="w")
                        for h2 in range(2):
                            nc.tensor.matmul(
                                ps_br[:, h2 * 512 : (h2 + 1) * 512],
                                ones[:],
                                bvp[:, h2 * 512 : (h2 + 1) * 512],
                                start=True,
                                stop=True,
                            )
                        nc.vector.tensor_copy(attn_acc[:], ps_br[:])

            # ============ Pass B: out = x @ Wv' + bv'
            with tc.tile_pool(name="pso", bufs=2, space="PSUM") as psopool:
                for ch in range(NCHUNK):
                    x0 = XT_OFF + (ch % 2) * (KT * CHUNK)
                    xt_sb = arena[:, x0 : x0 + KT * CHUNK]
                    nc.gpsimd.dma_start(
                        xt_sb.rearrange("p (t r) -> p t r", t=KT),
                        xt_v[:, :, ch * CHUNK : (ch + 1) * CHUNK],
                    )
                    for mi in range(MPC):
                        m = ch * MPC + mi
                        ps_o = psopool.tile([P, D], F32, tag="o")
                        for k in range(KT):
                            for h2 in range(2):
                                nc.tensor.matmul(
                                    ps_o[:, h2 * 512 : (h2 + 1) * 512],
                                    xt_sb[:, CHUNK * k + mi * P : CHUNK * k + (mi + 1) * P],
                                    arena[:, WVP_OFF + D * k + 512 * h2 : WVP_OFF + D * k + 512 * (h2 + 1)],
                                    start=(k == 0),
                                    stop=(k == KT - 1),
                                )
                        out_sb = opool.tile([P, D], F32, tag="osb")
                        if m == NSEQ // P - 1:
                            # split the last store so its DMA overlaps the
                            # second half's add (shaves the tail drain)
                            nc.vector.tensor_add(
                                out_sb[:, 0:512], ps_o[:, 0:512],
                                attn_acc[:, 0:512],
                            )
                            nc.scalar.dma_start(
                                out_d[m * P : (m + 1) * P, 0:512],
                                out_sb[:, 0:512],
                            )
                            nc.vector.tensor_add(
                                out_sb[:, 512:D], ps_o[:, 512:D],
                                attn_acc[:, 512:D],
                            )
                            nc.sync.dma_start(
                                out_d[m * P : (m + 1) * P, 512:D],
                                out_sb[:, 512:D],
                            )
                        else:
                            nc.vector.tensor_add(
                                out_sb[:], ps_o[:], attn_acc[:]
                            )
                            nc.scalar.dma_start(
                                out_d[m * P : (m + 1) * P, :], out_sb[:]
                            )

    nc.compile()
    return nc


def host_inputs(x, W_qkv, b_qkv):
    """Per-core input maps (host prep: transposes, packing, bias C)."""
    bf16 = ml_dtypes.bfloat16
    wvt = np.ascontiguousarray(
        W_qkv[:, 2 * D :].T.reshape(NPAIR, P, D).transpose(1, 0, 2)
        .reshape(P, NPAIR * D)
    ).astype(bf16)
    bv = np.ascontiguousarray(
        b_qkv[2 * D :].reshape(NPAIR, P).T
    ).astype(bf16)
    eye = np.eye(P, dtype=np.float32)
    ones = np.ones((1, P), np.float32)
    bq = b_qkv[:D]
    bk = b_qkv[D : 2 * D]

    in_maps = []
    for c in range(B):
        s = x[c].sum(axis=0, dtype=np.float64).astype(np.float32)
        sq = s @ W_qkv[:, :D]
        sk = s @ W_qkv[:, D : 2 * D]
        cpk = np.full((P, NPAIR * P), NEG, np.float32)
        for p in range(NPAIR):
            r = slice(P * p, P * (p + 1))
            sub = (
                np.outer(sq[r], bk[r])
                + np.outer(bq[r], sk[r])
                + float(NSEQ) * np.outer(bq[r], bk[r])
            )
            sub[:DH, DH:] = NEG
            sub[DH:, :DH] = NEG
            cpk[:, r] = sub
        in_maps.append(
            {
                "x": x[c],
                "xt": np.ascontiguousarray(x[c].T),
                "wk": np.ascontiguousarray(W_qkv[:, D : 2 * D]),
                "wq": np.ascontiguousarray(W_qkv[:, :D]),
                "wvt": wvt,
                "bv": bv,
                "cbias": cpk.astype(bf16),
                "eye": eye,
                "ones": ones,
            }
        )
    return in_maps


def kernel(x, W_qkv, b_qkv):
    global _LAST_RESULTS
    x = np.ascontiguousarray(x, dtype=np.float32)
    W_qkv = np.ascontiguousarray(W_qkv, dtype=np.float32)
    b_qkv = np.ascontiguousarray(b_qkv, dtype=np.float32)

    if "nc" not in _CACHE:
        _CACHE["nc"] = _build()
    nc = _CACHE["nc"]

    res = bass_utils.run_bass_kernel_spmd(
        nc, host_inputs(x, W_qkv, b_qkv), core_ids=list(range(B))
    )
    _LAST_RESULTS = res
    return np.stack([r["out"] for r in res.results], axis=0)
